# revision 28
# baseline (speedup 1.0000x reference)
"""Trainium2 Bass kernel for a 4-layer decoder transformer (B4 T1024 E1024 H16
hs64 F4096 V32000) on 8 NeuronCores.

Sharding: batch(4) x sequence-half(2). Core c handles batch b=c//2 and the
causal-interleaved token chunks CHUNKS[c%2]. The residual stream lives in
SBUF transposed (xT: [E, 512], E on partitions) so every matmul has its
contraction dim on partitions.

Cross-core exchange (v2): instead of AllGathering k|v (2MB in, 4MB out per
core), each core sends only its LN1 output hT (1MB) to its pair partner and
recomputes the partner's k/v locally. The exchange is a pair
ReduceScatter(add): each core scatters hT into the PARTNER's slot of agi
(own slot pre-zeroed once), so RS yields exactly the partner's hT -- half
the collective bytes of an AllGather of the same payload, and the
per-core-different slot offset is handled by an indirect DMA driven by a
host-supplied row-index input (keeps the program SPMD-uniform).

Attention indexes keys as (own/partner, chunk) triangular blocks: q chunk j
attends own chunks i<=j (diagonal tril-masked) and partner chunks i<=j
(diagonal 0/1-masked per core, input msk01). This is uniform across cores
and does the same 20 score blocks per head as the previous POSF scheme.

PSUM layout: tag "mm" = 4 rotating single-bank tiles, tag "mm4" = [128,
1024] 2-bank tiles (bufs=2) used for score batches so exp runs as 3 big ACT
ops per head. All matmul phases use 4-bank accumulation blocks so
consecutive blocks double-buffer. Elementwise psum drains alternate between
DVE and ACT to balance engine load. LN rstd = exp(-0.5*ln(var+eps)) keeps
every ACT call in the natural_log_exp table set (no table reloads).
"""

import numpy as np
import ml_dtypes

import concourse.bass as bass
import concourse.bacc as bacc
import concourse.mybir as mybir
import concourse.tile as tile
from concourse import bass_utils
from concourse.masks import make_identity

F32 = mybir.dt.float32
F32R = mybir.dt.float32r
BF16 = mybir.dt.bfloat16
I32 = mybir.dt.int32
AF = mybir.ActivationFunctionType
OP = mybir.AluOpType
P = 128

N_CORES = 8
PAIRS = [[0, 1], [2, 3], [4, 5], [6, 7]]


def _chunks(seq, n):
    seq = list(seq)
    return [seq[i:i + n] for i in range(0, len(seq), n)]


class Cfg:
    def __init__(self, B=4, T=1024, E=1024, H=16, HS=64, L=4, F=4096, V=32000):
        self.B, self.T, self.E, self.H, self.HS = B, T, E, H, HS
        self.L, self.F, self.V = L, F, V
        self.TC = T // 2                    # tokens per core
        self.NEC = E // P                   # E chunks (partition tiles)
        self.NTC = self.TC // P             # local token chunks
        self.NFC = F // P                   # FFN hidden chunks
        self.HPP = P // HS                  # heads per 128-partition tile
        self.NHP = (H * HS) // P            # head-pair tiles
        self.HP = HS + 1                    # augmented per-head stride in v
        self.scale = 1.0 / (E ** 0.5)
        self.hrow = self.NEC * self.TC      # per-partition hT row (4096)
        self.vchunks = []
        v0 = 0
        while v0 < V:
            self.vchunks.append((v0, min(512, V - v0)))
            v0 += 512
        self.vrow = H * self.HP
        # Causal-interleaved token chunks: core half h owns global 128-token
        # chunks CHUNKS[h] (in this local order). Both lists ascend, so
        # own-vs-own causality is exactly i<=j with tril on the diagonal,
        # and partner chunk i is valid for q chunk j iff
        # CHUNKS[partner][i] < CHUNKS[own][j] -- which holds for all i<j and
        # alternates on the diagonal (host input msk01).
        self.CHUNKS = [[0, 3, 4, 7], [1, 2, 5, 6]]


def build_program(c: Cfg, reps: int = 1, ablate=()):
    # Every ACT call here uses only {Exp, Ln, Relu, Copy, Identity}, all of
    # which live in the natural_log_exp_and_others table set. The stock
    # table chooser maps Exp->exp_and_others and Ln->natural_log, inserting
    # a ~1.3us table load per switch (2 per layernorm). Restrict the
    # eligible sets (ids preserved) during this build so one load serves
    # the whole program.
    import concourse.bacc as _bacc_mod
    _orig_tables = _bacc_mod.get_activation_tables

    def _only_ln_exp(arch):
        tabs = _orig_tables(arch)
        return {name: (s if name == "natural_log_exp_and_others" else set())
                for name, s in tabs.items()}

    _bacc_mod.get_activation_tables = _only_ln_exp
    try:
        return _build_program_inner(c, reps, ablate)
    finally:
        _bacc_mod.get_activation_tables = _orig_tables


def _build_program_inner(c: Cfg, reps: int = 1, ablate=()):
    nc = bacc.Bacc("TRN2", target_bir_lowering=False, debug=False,
                   num_devices=N_CORES)

    # ---- DRAM I/O ----
    dt_ = nc.dram_tensor
    idx_t = dt_("idx", [c.TC], I32, kind="ExternalInput").ap()
    temb_t = dt_("temb", [c.V, c.E], BF16, kind="ExternalInput").ap()
    posT_t = dt_("posT", [c.E, c.TC], F32, kind="ExternalInput").ap()
    wqkv_t = dt_("wqkv", [c.L, c.E, 3 * c.H * c.HS], BF16,
                 kind="ExternalInput").ap()
    wo_t = dt_("wo", [c.L, c.E, c.E], BF16, kind="ExternalInput").ap()
    bo_t = dt_("bo", [c.L, c.E], F32, kind="ExternalInput").ap()
    ln1g_t = dt_("ln1g", [c.L, c.E], F32, kind="ExternalInput").ap()
    ln1b_t = dt_("ln1b", [c.L, c.E], F32, kind="ExternalInput").ap()
    ln2g_t = dt_("ln2g", [c.L, c.E], F32, kind="ExternalInput").ap()
    ln2b_t = dt_("ln2b", [c.L, c.E], F32, kind="ExternalInput").ap()
    w1_t = dt_("w1", [c.L, c.E, c.F], BF16, kind="ExternalInput").ap()
    b1_t = dt_("b1", [c.L, c.F], F32, kind="ExternalInput").ap()
    w2_t = dt_("w2", [c.L, c.F, c.E], BF16, kind="ExternalInput").ap()
    b2_t = dt_("b2", [c.L, c.E], F32, kind="ExternalInput").ap()
    lnfg_t = dt_("lnfg", [c.E], F32, kind="ExternalInput").ap()
    lnfb_t = dt_("lnfb", [c.E], F32, kind="ExternalInput").ap()
    wh_t = dt_("wh", [c.E, c.V], BF16, kind="ExternalInput").ap()
    tril_t = dt_("tril", [P, P], BF16, kind="ExternalInput").ap()
    msk01_t = dt_("msk01", [P, c.NTC], F32, kind="ExternalInput").ap()
    pidx_t = dt_("pidx", [P], I32, kind="ExternalInput").ap()
    oidx_t = dt_("oidx", [P], I32, kind="ExternalInput").ap()
    # logits written bf16; host upcasts to f32 and adds bh there
    out_t = dt_("out", [c.TC, c.V], BF16, kind="ExternalOutput").ap()

    with tile.TileContext(nc) as tc:
        with tc.tile_pool(name="sb", bufs=1) as sb, \
             tc.tile_pool(name="wpool", bufs=4) as wpool, \
             tc.tile_pool(name="xpool", bufs=2) as xpool, \
             tc.tile_pool(name="ps", bufs=4, space="PSUM") as ps, \
             tc.tile_pool(name="dram", bufs=1, space="DRAM") as dram:

            def psum(shape=None, dtype=F32, name="mm"):
                return ps.tile(shape or [P, 512], dtype, tag="mm", name=name)

            def psum4(name="mm4"):
                return ps.tile([P, 2 * 512], F32, tag="mm4", name=name,
                               bufs=2)

            # Projection phases need 4 [P,512] accumulators per output
            # group; alternating groups between the mm banks and the mm4
            # banks (idle outside attention) lets group g+1's matmuls run
            # while group g's psum->sbuf copies drain.
            pgc = [0]

            def psum_group(names):
                pgc[0] += 1
                if pgc[0] % 2 == 0:
                    return [psum(name=nm) for nm in names]
                a = psum4(name="pgA")
                b = psum4(name="pgB")
                views = [a[:, 0:512], a[:, 512:1024],
                         b[:, 0:512], b[:, 512:1024]]
                return views[:len(names)]

            # psum->sbuf drains alternate DVE/ACT to balance engine load
            cpc = [0]

            def drain_copy(out, in_):
                cpc[0] += 1
                if cpc[0] % 2 == 0:
                    nc.scalar.copy(out, in_)
                else:
                    nc.vector.tensor_copy(out=out, in_=in_)

            # ---- constants ----
            ones_bf = sb.tile([P, 1], BF16, tag="ones_bf", name="ones_bf")
            nc.vector.memset(ones_bf[:], 1.0)
            ident = sb.tile([P, P], BF16, tag="ident", name="ident")
            make_identity(nc, ident[:])
            tril_sb = sb.tile([P, P], BF16, tag="tril", name="tril_sb")
            nc.sync.dma_start(tril_sb[:], tril_t)
            msk01_sb = sb.tile([P, c.NTC], F32, tag="msk01", name="msk01_sb")
            nc.sync.dma_start(msk01_sb[:], msk01_t)
            pidx_sb = sb.tile([P, 1], I32, tag="pidx", name="pidx_sb")
            nc.sync.dma_start(pidx_sb[:], pidx_t.rearrange("(p o) -> p o", o=1))
            oidx_sb = sb.tile([P, 1], I32, tag="oidx", name="oidx_sb")
            nc.sync.dma_start(oidx_sb[:], oidx_t.rearrange("(p o) -> p o", o=1))
            idx_sb = sb.tile([P, c.NTC], I32, tag="idx", name="idx_sb")
            nc.sync.dma_start(idx_sb[:], idx_t.rearrange("(tc p) -> p tc", p=P))

            # ---- collective staging: agi [2P, hrow]; own slot rows are
            # zeroed once so the pair ReduceScatter(add) yields exactly the
            # partner's hT in ago.
            agi_d = dram.tile([2 * P, c.hrow], BF16, tag="agi", name="agi")
            ago_d = dram.tile([P * c.hrow], BF16, tag="ago", name="ago")
            zT = sb.tile([P, c.hrow], BF16, tag="zT", name="zT")
            nc.vector.memset(zT[:], 0.0)
            nc.gpsimd.indirect_dma_start(
                out=agi_d[:], out_offset=bass.IndirectOffsetOnAxis(
                    ap=oidx_sb[:], axis=0),
                in_=zT[:], in_offset=None)

            for _rep in range(reps):
                # ---- residual stream xT[e, t] (f32), seeded with pos^T ----
                xT = sb.tile([P, c.NEC, c.TC], F32, tag="xT", name="xT")
                nc.sync.dma_start(
                    xT[:], posT_t.rearrange("(ec p) t -> p ec t", p=P))

                # ---- embedding gather + transpose ----
                for tcb in range(c.NTC):
                    emb = xpool.tile([P, c.E], BF16, tag="emb", name="emb")
                    if "gather" in ablate:
                        nc.sync.dma_start(emb[:],
                                          temb_t[tcb * P:(tcb + 1) * P, :])
                    else:
                        nc.gpsimd.indirect_dma_start(
                            out=emb[:], out_offset=None, in_=temb_t,
                            in_offset=bass.IndirectOffsetOnAxis(
                                ap=idx_sb[:, tcb:tcb + 1], axis=0))
                    for ec in range(c.NEC):
                        tps = psum([P, P], BF16, name="tps")
                        nc.tensor.transpose(
                            out=tps[:], in_=emb[:, ec * P:(ec + 1) * P],
                            identity=ident[:])
                        tpf = xpool.tile([P, P], F32, tag="tpf", name="tpf")
                        nc.vector.tensor_copy(out=tpf[:], in_=tps[:])
                        sl = xT[:, ec, tcb * P:(tcb + 1) * P]
                        nc.vector.tensor_tensor(out=sl, in0=sl, in1=tpf[:],
                                                op=OP.add)

                # ---- layernorm: xT -> out_bf (bf16 [P, NEC, TC]) ----
                # rstd = exp(-0.5*ln(var+eps)) stays in the exp/ln ACT set.
                def layernorm(xT, g_dram, b_dram, out_bf):
                    gb = sb.tile([P, 2 * c.NEC], F32, tag="gains", name="gb",
                                 bufs=2)
                    nc.sync.dma_start(
                        gb[:, 0:c.NEC], g_dram.rearrange("(ec p) -> p ec", p=P))
                    nc.sync.dma_start(
                        gb[:, c.NEC:], b_dram.rearrange("(ec p) -> p ec", p=P))
                    sum_ps = psum([1, c.TC], name="ln_sum")
                    sq_ps = psum([1, c.TC], name="ln_sq")
                    for ec in range(c.NEC):
                        xbf = xpool.tile([P, c.TC], BF16, tag="xbf", name="xbf")
                        nc.vector.tensor_copy(out=xbf[:], in_=xT[:, ec, :])
                        nc.tensor.matmul(out=sum_ps[:], lhsT=ones_bf[:],
                                         rhs=xbf[:], start=(ec == 0),
                                         stop=(ec == c.NEC - 1))
                        xsq = xpool.tile([P, c.TC], BF16, tag="xsq", name="xsq")
                        if ec % 2 == 0:
                            nc.scalar.activation(xsq[:], xbf[:], AF.Square)
                        else:
                            nc.vector.tensor_tensor(out=xsq[:], in0=xbf[:],
                                                    in1=xbf[:], op=OP.mult)
                        nc.tensor.matmul(out=sq_ps[:], lhsT=ones_bf[:],
                                         rhs=xsq[:], start=(ec == 0),
                                         stop=(ec == c.NEC - 1))
                    stats = xpool.tile([1, 3 * c.TC], F32, tag="stats",
                                       name="stats", bufs=1)
                    mean = stats[:, 0:c.TC]
                    var = stats[:, c.TC:2 * c.TC]
                    rstd = stats[:, 2 * c.TC:]
                    inv_e = 1.0 / c.E
                    nc.scalar.mul(mean, sum_ps[:], inv_e)
                    nc.scalar.mul(var, sq_ps[:], inv_e)
                    m2 = xpool.tile([1, c.TC], F32, tag="m2", name="m2")
                    nc.vector.tensor_tensor(out=m2[:], in0=mean, in1=mean,
                                            op=OP.mult)
                    nc.vector.tensor_tensor(out=var, in0=var, in1=m2[:],
                                            op=OP.subtract)
                    nc.vector.tensor_scalar_add(out=var, in0=var, scalar1=1e-5)
                    nc.scalar.activation(var, var, AF.Ln)
                    nc.scalar.activation(rstd, var, AF.Exp, scale=-0.5)
                    mrb = xpool.tile([P, 2, c.TC], F32, tag="mrb",
                                     name="mrb", bufs=1)
                    nc.gpsimd.partition_broadcast(mrb[:, 0, :], mean)
                    nc.gpsimd.partition_broadcast(mrb[:, 1, :], rstd)
                    for ec in range(c.NEC):
                        tmp = xpool.tile([P, c.TC], F32, tag="lntmp",
                                         name="lntmp")
                        nc.vector.tensor_tensor(out=tmp[:], in0=xT[:, ec, :],
                                                in1=mrb[:, 0, :],
                                                op=OP.subtract)
                        nc.vector.tensor_tensor(out=tmp[:], in0=tmp[:],
                                                in1=mrb[:, 1, :], op=OP.mult)
                        nc.vector.tensor_scalar(
                            out=out_bf[:, ec, :], in0=tmp[:],
                            scalar1=gb[:, ec:ec + 1],
                            scalar2=gb[:, c.NEC + ec:c.NEC + ec + 1],
                            op0=OP.mult, op1=OP.add)

                # qk-style projection: dst[feat_chunk, tok] from src hT-like
                def qk_proj(which, dst, src, l):
                    col0 = which * c.H * c.HS
                    for fcs in _chunks(range(c.NHP), 4):
                        pss = dict(zip(fcs, psum_group(
                            [f"qk{fc}" for fc in fcs])))
                        w = len(fcs) * P
                        wt = wpool.tile([P, c.NEC, w], BF16,
                                        tag="wblk", name="wt")
                        nc.sync.dma_start(
                            wt[:],
                            wqkv_t[l, :, col0 + fcs[0] * P:
                                   col0 + fcs[0] * P + w]
                            .rearrange("(ec p) w -> p ec w", p=P))
                        for ec in range(c.NEC):
                            for j, fc in enumerate(fcs):
                                nc.tensor.matmul(
                                    out=pss[fc][:, :c.TC],
                                    lhsT=wt[:, ec, j * P:(j + 1) * P],
                                    rhs=src[:, ec, :],
                                    start=(ec == 0),
                                    stop=(ec == c.NEC - 1))
                        for fc in fcs:
                            drain_copy(dst[:, fc, :], pss[fc][:, :c.TC])

                # v projection (natural [tok, vrow] layout, ones col per head)
                def v_proj(dst, src, l):
                    vw = min(512, c.H * c.HS)
                    nvh = (c.H * c.HS) // vw
                    hs_per_vh = vw // c.HS
                    col0 = 2 * c.H * c.HS
                    vjobs = [(tcb, vh) for tcb in range(c.NTC)
                             for vh in range(nvh)]
                    for grp in _chunks(vjobs, 4):
                        pss = dict(zip(grp, psum_group(
                            [f"v{j[0]}_{j[1]}" for j in grp])))
                        wts = {}
                        for vh in sorted({vh for _, vh in grp}):
                            wt = wpool.tile([P, c.NEC, vw], BF16,
                                            tag="wblk", name="wt")
                            nc.sync.dma_start(
                                wt[:],
                                wqkv_t[l, :, col0 + vh * vw:
                                       col0 + (vh + 1) * vw]
                                .rearrange("(ec p) w -> p ec w", p=P))
                            wts[vh] = wt
                        for ec in range(c.NEC):
                            for tcb, vh in grp:
                                nc.tensor.matmul(
                                    out=pss[(tcb, vh)][:, :vw],
                                    lhsT=src[:, ec, tcb * P:(tcb + 1) * P],
                                    rhs=wts[vh][:, ec, :],
                                    start=(ec == 0), stop=(ec == c.NEC - 1))
                        for tcb, vh in grp:
                            for hh in range(hs_per_vh):
                                h = vh * hs_per_vh + hh
                                drain_copy(
                                    dst[:, tcb, h * c.HP:h * c.HP + c.HS],
                                    pss[(tcb, vh)][:, hh * c.HS:
                                                   (hh + 1) * c.HS])

                # ============ layers ============
                for l in range(c.L):
                    hT = sb.tile([P, c.NEC, c.TC], BF16, tag="hT", name="hT")
                    layernorm(xT, ln1g_t[l], ln1b_t[l], hT)

                    # ---- pair exchange of hT via zero-slot ReduceScatter ----
                    if "coll" not in ablate:
                        nc.gpsimd.indirect_dma_start(
                            out=agi_d[:],
                            out_offset=bass.IndirectOffsetOnAxis(
                                ap=pidx_sb[:], axis=0),
                            in_=hT[:].rearrange("p a b -> p (a b)"),
                            in_offset=None)
                        nc.gpsimd.collective_compute(
                            "ReduceScatter", OP.add, replica_groups=PAIRS,
                            ins=[agi_d[:].rearrange("a b -> (a b)")],
                            outs=[ago_d[:]])
                    else:
                        nc.sync.dma_start(
                            ago_d[:].rearrange("(p w) -> p w", p=P),
                            hT[:].rearrange("p a b -> p (a b)"))

                    # ---- own-half projections overlap the collective ----
                    kf_own = sb.tile([P, c.NHP, c.TC], BF16, tag="kfo",
                                     name="kf_own")
                    qT = sb.tile([P, c.NHP, c.TC], BF16, tag="qT", name="qT")
                    vf_own = sb.tile([P, c.NTC, c.vrow], BF16, tag="vfo",
                                     name="vf_own")
                    vf_par = sb.tile([P, c.NTC, c.vrow], BF16, tag="vfp",
                                     name="vf_par")
                    for h in range(c.H):
                        for v_ in (vf_own, vf_par):
                            nc.vector.memset(
                                v_[:, :, h * c.HP + c.HS:
                                   h * c.HP + c.HS + 1], 1.0)
                    if "qkv" in ablate:
                        nc.vector.memset(qT[:], 0.0078125)
                        nc.vector.memset(kf_own[:], 0.0078125)
                    else:
                        qk_proj(1, kf_own, hT, l)
                        qk_proj(0, qT, hT, l)
                        v_proj(vf_own, hT, l)

                    # ---- partner h arrives; recompute partner k/v ----
                    # hf load goes through gpsimd (SWDGE): its wait on the
                    # collective must not head-of-line-block the sync DGE
                    # queue, where it would stall the partner weight
                    # prefetches that have no ago dependency.
                    hf = sb.tile([P, c.NEC, c.TC], BF16, tag="hfp",
                                 name="hfp")
                    nc.gpsimd.dma_start(
                        hf[:], ago_d[:].rearrange("(p ec t) -> p ec t",
                                                  p=P, ec=c.NEC))
                    kf_par = sb.tile([P, c.NHP, c.TC], BF16, tag="kfp",
                                     name="kf_par")
                    if "qkv" in ablate:
                        nc.vector.memset(kf_par[:], 0.0078125)
                    else:
                        qk_proj(1, kf_par, hf, l)
                        v_proj(vf_par, hf, l)

                    # ---- attention ----
                    # q chunk j attends own chunks i<=j (diag: tril) and
                    # partner chunks i<=j (diag: msk01 0/1 per core).
                    attT = sb.tile([P, c.NHP, c.TC], BF16, tag="attT",
                                   name="attT")
                    if "attn" in ablate:
                        nc.vector.memset(attT[:], 0.00390625)
                    for h in range(0 if "attn" in ablate else c.H):
                        hp, hb = divmod(h, c.HPP)
                        p0 = hb * c.HS
                        att_ps = psum(name="att_ps")
                        for grp in ([0, 1], [2], [3]):
                            offs, o = {}, 0
                            for j in grp:
                                offs[j] = o
                                o += 2 * (j + 1) * P
                            s4 = psum4(name="s4")
                            for j in grp:
                                q0 = j * P
                                for si, kfx in ((0, kf_own), (1, kf_par)):
                                    for i in range(j + 1):
                                        col = offs[j] + (si * (j + 1) + i) * P
                                        nc.tensor.matmul(
                                            out=s4[:, col:col + P],
                                            lhsT=kfx[p0:p0 + c.HS, hp,
                                                     i * P:(i + 1) * P],
                                            rhs=qT[p0:p0 + c.HS, hp,
                                                   q0:q0 + P],
                                            start=True, stop=True)
                            ex = xpool.tile([P, 8 * P], BF16, tag="ex",
                                            name="ex", bufs=3)
                            if "exp" in ablate:
                                nc.vector.tensor_copy(
                                    out=ex[:, :o], in_=s4[:, :o])
                            else:
                                nc.scalar.activation(
                                    ex[:, :o], s4[:, :o],
                                    AF.Exp, scale=c.scale)
                                for j in grp:
                                    od = offs[j] + j * P
                                    nc.vector.tensor_tensor(
                                        out=ex[:, od:od + P],
                                        in0=ex[:, od:od + P],
                                        in1=tril_sb[:], op=OP.mult)
                                    pd = offs[j] + (2 * j + 1) * P
                                    nc.vector.tensor_scalar_mul(
                                        ex[:, pd:pd + P], ex[:, pd:pd + P],
                                        msk01_sb[:, j:j + 1])
                            for j in grp:
                                q0 = j * P
                                nblk = 2 * (j + 1)
                                if "av" in ablate:
                                    nc.vector.memset(
                                        att_ps[:c.HP, q0:q0 + P], 0.0078125)
                                    continue
                                bi = 0
                                for si, vfx in ((0, vf_own), (1, vf_par)):
                                    for i in range(j + 1):
                                        col = offs[j] + bi * P
                                        nc.tensor.matmul(
                                            out=att_ps[:c.HP, q0:q0 + P],
                                            lhsT=vfx[:, i,
                                                     h * c.HP:(h + 1) * c.HP],
                                            rhs=ex[:, col:col + P],
                                            start=(bi == 0),
                                            stop=(bi == nblk - 1))
                                        bi += 1
                        rec = xpool.tile([1, c.TC], F32, tag="rec",
                                         name="rec", bufs=2)
                        nc.vector.reciprocal(rec[:], att_ps[c.HS:c.HP, :c.TC])
                        recb = xpool.tile([c.HS, c.TC], F32, tag="recb",
                                          name="recb", bufs=2)
                        nc.gpsimd.partition_broadcast(recb[:], rec[:])
                        nc.vector.tensor_tensor(
                            out=attT[p0:p0 + c.HS, hp, :],
                            in0=att_ps[:c.HS, :c.TC],
                            in1=recb[:], op=OP.mult)

                    # ---- Wo projection + bo + residual ----
                    bob = sb.tile([P, c.NEC], F32, tag="bob", name="bob",
                                  bufs=2)
                    nc.sync.dma_start(
                        bob[:], bo_t[l].rearrange("(ec p) -> p ec", p=P))
                    for eos in _chunks(range(c.NEC), 4):
                        pss = dict(zip(eos, psum_group(
                            [f"wo{eo}" for eo in eos])))
                        w = len(eos) * P
                        wt = wpool.tile([P, c.NEC, w], BF16,
                                        tag="wblk", name="wt")
                        nc.sync.dma_start(
                            wt[:], wo_t[l, :, eos[0] * P:eos[0] * P + w]
                            .rearrange("(ec p) w -> p ec w", p=P))
                        for ec in range(c.NEC):
                            for j, eo in enumerate(eos):
                                nc.tensor.matmul(
                                    out=pss[eo][:, :c.TC],
                                    lhsT=wt[:, ec, j * P:(j + 1) * P],
                                    rhs=attT[:, ec, :],
                                    start=(ec == 0), stop=(ec == c.NEC - 1))
                        for eo in eos:
                            tmp = xpool.tile([P, c.TC], F32, tag="lntmp",
                                             name="rtmp")
                            if eo % 2 == 0:
                                nc.scalar.add(tmp[:], pss[eo][:, :c.TC],
                                              bob[:, eo:eo + 1])
                            else:
                                nc.vector.tensor_scalar_add(
                                    out=tmp[:], in0=pss[eo][:, :c.TC],
                                    scalar1=bob[:, eo:eo + 1])
                            nc.vector.tensor_tensor(
                                out=xT[:, eo, :], in0=xT[:, eo, :],
                                in1=tmp[:], op=OP.add)

                    # ---- LN2 + FFN ----
                    h2T = sb.tile([P, c.NEC, c.TC], BF16, tag="hT", name="h2T")
                    layernorm(xT, ln2g_t[l], ln2b_t[l], h2T)

                    b1b = sb.tile([P, c.NFC], F32, tag="b1b", name="b1b",
                                  bufs=2)
                    nc.sync.dma_start(
                        b1b[:], b1_t[l].rearrange("(fc p) -> p fc", p=P))
                    uT = sb.tile([P, c.NFC, c.TC], BF16, tag="uT", name="uT")
                    if "ffn" in ablate:
                        nc.vector.memset(uT[:], 0.0078125)
                    for fcs in ([] if "ffn" in ablate
                                else _chunks(range(c.NFC), 4)):
                        pss = dict(zip(fcs, psum_group(
                            [f"u{fc}" for fc in fcs])))
                        w = len(fcs) * P
                        wt = wpool.tile([P, c.NEC, w], BF16,
                                        tag="wblk", name="wt")
                        nc.sync.dma_start(
                            wt[:], w1_t[l, :, fcs[0] * P:fcs[0] * P + w]
                            .rearrange("(ec p) w -> p ec w", p=P))
                        for ec in range(c.NEC):
                            for j, fc in enumerate(fcs):
                                nc.tensor.matmul(
                                    out=pss[fc][:, :c.TC],
                                    lhsT=wt[:, ec, j * P:(j + 1) * P],
                                    rhs=h2T[:, ec, :],
                                    start=(ec == 0), stop=(ec == c.NEC - 1))
                        for fc in fcs:
                            # relu(x + b1): alternate ACT / DVE
                            if fc % 2 == 0:
                                nc.scalar.activation(
                                    uT[:, fc, :], pss[fc][:, :c.TC],
                                    AF.Relu, bias=b1b[:, fc:fc + 1])
                            else:
                                nc.vector.tensor_scalar(
                                    out=uT[:, fc, :], in0=pss[fc][:, :c.TC],
                                    scalar1=b1b[:, fc:fc + 1], scalar2=0.0,
                                    op0=OP.add, op1=OP.max)

                    b2b = sb.tile([P, c.NEC], F32, tag="bob", name="b2b",
                                  bufs=2)
                    nc.sync.dma_start(
                        b2b[:], b2_t[l].rearrange("(ec p) -> p ec", p=P))
                    for eos in ([] if "ffn" in ablate
                                else _chunks(range(c.NEC), 4)):
                        pss = dict(zip(eos, psum_group(
                            [f"y{eo}" for eo in eos])))
                        w = len(eos) * P
                        for kcs in _chunks(range(c.NFC), 8):
                            wt = wpool.tile([P, len(kcs), w], BF16,
                                            tag="wblk", name="wt")
                            nc.sync.dma_start(
                                wt[:], w2_t[l, kcs[0] * P:
                                            (kcs[-1] + 1) * P,
                                            eos[0] * P:eos[0] * P + w]
                                .rearrange("(kc p) w -> p kc w", p=P))
                            for ki, kc in enumerate(kcs):
                                for j, eo in enumerate(eos):
                                    nc.tensor.matmul(
                                        out=pss[eo][:, :c.TC],
                                        lhsT=wt[:, ki, j * P:(j + 1) * P],
                                        rhs=uT[:, kc, :],
                                        start=(kc == 0),
                                        stop=(kc == c.NFC - 1))
                        for eo in eos:
                            tmp = xpool.tile([P, c.TC], F32, tag="lntmp",
                                             name="ytmp")
                            if eo % 2 == 0:
                                nc.scalar.add(tmp[:], pss[eo][:, :c.TC],
                                              b2b[:, eo:eo + 1])
                            else:
                                nc.vector.tensor_scalar_add(
                                    out=tmp[:], in0=pss[eo][:, :c.TC],
                                    scalar1=b2b[:, eo:eo + 1])
                            nc.vector.tensor_tensor(
                                out=xT[:, eo, :], in0=xT[:, eo, :],
                                in1=tmp[:], op=OP.add)

                # ============ final LN + lm_head ============
                xlnT = sb.tile([P, c.NEC, c.TC], BF16, tag="hT", name="xlnT")
                layernorm(xT, lnfg_t, lnfb_t, xlnT)

                vcs = [] if "lmhead" in ablate else c.vchunks
                for v0, wv in vcs:
                    pss = dict(zip(range(c.NTC), psum_group(
                        [f"lg{t}" for t in range(c.NTC)])))
                    wt = wpool.tile([P, c.NEC, 512], BF16, tag="wblk",
                                    name="wt")
                    nc.sync.dma_start(
                        wt[:, :, :wv], wh_t[:, v0:v0 + wv]
                        .rearrange("(ec p) w -> p ec w", p=P))
                    for ec in range(c.NEC):
                        for tcb in range(c.NTC):
                            nc.tensor.matmul(
                                out=pss[tcb][:, :wv],
                                lhsT=xlnT[:, ec, tcb * P:(tcb + 1) * P],
                                rhs=wt[:, ec, :wv],
                                start=(ec == 0), stop=(ec == c.NEC - 1))
                    lg = xpool.tile([P, c.NTC, 512], BF16, tag="lg",
                                    name="lg", bufs=2)
                    for tcb in range(c.NTC):
                        drain_copy(lg[:, tcb, :wv], pss[tcb][:, :wv])
                    nc.sync.dma_start(
                        out_t[:, v0:v0 + wv]
                        .rearrange("(tcb p) w -> p tcb w", p=P),
                        lg[:, :, :wv])

    nc.compile()
    return nc


# ----------------------------------------------------------------------------
# host side
# ----------------------------------------------------------------------------

def prep_inputs(c: Cfg, inputs):
    """Build the 8 per-core input maps from the full model inputs."""
    bf = ml_dtypes.bfloat16
    f32 = np.float32

    idx = np.asarray(inputs["idx"]).astype(np.int32)
    temb = np.asarray(inputs["tok_emb"], f32).astype(bf)
    pos = np.asarray(inputs["pos_emb"], f32)
    Wq, Wk, Wv = (np.asarray(inputs[k], f32) for k in ("Wq", "Wk", "Wv"))
    EHH = c.H * c.HS
    wqkv = np.ascontiguousarray(np.concatenate(
        [w.transpose(0, 2, 1, 3).reshape(c.L, c.E, EHH)
         for w in (Wq, Wk, Wv)], axis=2).astype(bf))

    kk = np.arange(P)[:, None]
    qq = np.arange(P)[None, :]
    tril = np.ascontiguousarray((kk <= qq).astype(bf))

    shared = {
        "temb": temb, "wqkv": wqkv,
        "wo": np.asarray(inputs["Wo"], f32).astype(bf),
        "w1": np.asarray(inputs["W1"], f32).astype(bf),
        "w2": np.asarray(inputs["W2"], f32).astype(bf),
        "wh": np.asarray(inputs["Wh"], f32).astype(bf),
        "bo": np.asarray(inputs["bo"], f32),
        "ln1g": np.asarray(inputs["ln1_g"], f32),
        "ln1b": np.asarray(inputs["ln1_b"], f32),
        "ln2g": np.asarray(inputs["ln2_g"], f32),
        "ln2b": np.asarray(inputs["ln2_b"], f32),
        "b1": np.asarray(inputs["b1"], f32),
        "b2": np.asarray(inputs["b2"], f32),
        "lnfg": np.asarray(inputs["lnf_g"], f32),
        "lnfb": np.asarray(inputs["lnf_b"], f32),
        "tril": tril,
    }

    in_maps = []
    for core in range(N_CORES):
        b, half = divmod(core, 2)
        own = c.CHUNKS[half]
        par = c.CHUNKS[1 - half]
        tok = np.concatenate([np.arange(g * P, (g + 1) * P) for g in own])
        # partner diagonal block (q chunk j vs partner chunk j) validity
        msk01 = np.zeros((P, c.NTC), np.float32)
        for j in range(c.NTC):
            msk01[:, j] = 1.0 if par[j] < own[j] else 0.0
        in_maps.append(dict(
            shared,
            idx=np.ascontiguousarray(idx[b, tok]),
            posT=np.ascontiguousarray(pos[tok].T),
            msk01=np.ascontiguousarray(msk01),
            pidx=((1 - half) * P + np.arange(P)).astype(np.int32),
            oidx=(half * P + np.arange(P)).astype(np.int32),
        ))
    return in_maps


_CACHE = {}


def _get_program():
    if "nc" not in _CACHE:
        _CACHE["cfg"] = Cfg()
        _CACHE["nc"] = build_program(_CACHE["cfg"])
    return _CACHE["nc"], _CACHE["cfg"]


def kernel(**inputs) -> np.ndarray:
    nc, c = _get_program()
    in_maps = prep_inputs(c, inputs)
    res = bass_utils.run_bass_kernel_spmd(
        nc, in_maps, core_ids=list(range(N_CORES)))
    out = np.empty((c.B, c.T, c.V), np.float32)
    for core in range(N_CORES):
        b, half = divmod(core, 2)
        o = res.results[core]["out"].astype(np.float32)
        for i, g in enumerate(c.CHUNKS[half]):
            out[b, g * P:(g + 1) * P] = o[i * P:(i + 1) * P]
    out += np.asarray(inputs["bh"], np.float32)
    return out


# revision 33
# speedup vs baseline: 1.2838x; 1.2838x over previous
"""Trainium2 Bass kernel for a 4-layer decoder transformer (B4 T1024 E1024 H16
hs64 F4096 V32000) on 8 NeuronCores.

Sharding: batch(4) x sequence-half(2). Core c handles batch b=c//2 and the
causal-interleaved token chunks CHUNKS[c%2]. The residual stream lives in
SBUF transposed (xT: [E, 512], E on partitions) so every matmul has its
contraction dim on partitions.

Cross-core exchange (v2): instead of AllGathering k|v (2MB in, 4MB out per
core), each core sends only its LN1 output hT (1MB) to its pair partner and
recomputes the partner's k/v locally. The exchange is a pair
ReduceScatter(add): each core scatters hT into the PARTNER's slot of agi
(own slot pre-zeroed once), so RS yields exactly the partner's hT -- half
the collective bytes of an AllGather of the same payload, and the
per-core-different slot offset is handled by an indirect DMA driven by a
host-supplied row-index input (keeps the program SPMD-uniform).

Attention indexes keys as (own/partner, chunk) triangular blocks: q chunk j
attends own chunks i<=j (diagonal tril-masked) and partner chunks i<=j
(diagonal 0/1-masked per core, input msk01). This is uniform across cores
and does the same 20 score blocks per head as the previous POSF scheme.

PSUM layout: tag "mm" = 4 rotating single-bank tiles, tag "mm4" = [128,
1024] 2-bank tiles (bufs=2) used for score batches so exp runs as 3 big ACT
ops per head. All matmul phases use 4-bank accumulation blocks so
consecutive blocks double-buffer. Elementwise psum drains alternate between
DVE and ACT to balance engine load. LN rstd = exp(-0.5*ln(var+eps)) keeps
every ACT call in the natural_log_exp table set (no table reloads).
"""

import numpy as np
import ml_dtypes

import concourse.bass as bass
import concourse.bacc as bacc
import concourse.mybir as mybir
import concourse.tile as tile
from concourse import bass_utils
from concourse.masks import make_identity

F32 = mybir.dt.float32
F32R = mybir.dt.float32r
BF16 = mybir.dt.bfloat16
I32 = mybir.dt.int32
AF = mybir.ActivationFunctionType
OP = mybir.AluOpType
P = 128

N_CORES = 8
PAIRS = [[0, 1], [2, 3], [4, 5], [6, 7]]


def _chunks(seq, n):
    seq = list(seq)
    return [seq[i:i + n] for i in range(0, len(seq), n)]


class Cfg:
    def __init__(self, B=4, T=1024, E=1024, H=16, HS=64, L=4, F=4096, V=32000):
        self.B, self.T, self.E, self.H, self.HS = B, T, E, H, HS
        self.L, self.F, self.V = L, F, V
        self.TC = T // 2                    # tokens per core
        self.NEC = E // P                   # E chunks (partition tiles)
        self.NTC = self.TC // P             # local token chunks
        self.NFC = F // P                   # FFN hidden chunks
        self.HPP = P // HS                  # heads per 128-partition tile
        self.NHP = (H * HS) // P            # head-pair tiles
        self.HP = HS + 1                    # augmented per-head stride in v
        self.scale = 1.0 / (E ** 0.5)
        self.hrow = self.NEC * self.TC      # per-partition hT row (4096)
        self.vchunks = []
        v0 = 0
        while v0 < V:
            self.vchunks.append((v0, min(512, V - v0)))
            v0 += 512
        self.vrow = H * self.HP
        # Causal-interleaved token chunks: core half h owns global 128-token
        # chunks CHUNKS[h] (in this local order). Both lists ascend, so
        # own-vs-own causality is exactly i<=j with tril on the diagonal,
        # and partner chunk i is valid for q chunk j iff
        # CHUNKS[partner][i] < CHUNKS[own][j] -- which holds for all i<j and
        # alternates on the diagonal (host input msk01).
        self.CHUNKS = [[0, 3, 4, 7], [1, 2, 5, 6]]


def build_program(c: Cfg, reps: int = 1, ablate=()):
    # Every ACT call here uses only {Exp, Ln, Relu, Copy, Identity}, all of
    # which live in the natural_log_exp_and_others table set. The stock
    # table chooser maps Exp->exp_and_others and Ln->natural_log, inserting
    # a ~1.3us table load per switch (2 per layernorm). Restrict the
    # eligible sets (ids preserved) during this build so one load serves
    # the whole program.
    import concourse.bacc as _bacc_mod
    _orig_tables = _bacc_mod.get_activation_tables

    def _only_ln_exp(arch):
        tabs = _orig_tables(arch)
        return {name: (s if name == "natural_log_exp_and_others" else set())
                for name, s in tabs.items()}

    _bacc_mod.get_activation_tables = _only_ln_exp
    try:
        return _build_program_inner(c, reps, ablate)
    finally:
        _bacc_mod.get_activation_tables = _orig_tables


def _build_program_inner(c: Cfg, reps: int = 1, ablate=()):
    nc = bacc.Bacc("TRN2", target_bir_lowering=False, debug=False,
                   num_devices=N_CORES)

    # ---- DRAM I/O ----
    dt_ = nc.dram_tensor
    idx_t = dt_("idx", [c.TC], I32, kind="ExternalInput").ap()
    temb_t = dt_("temb", [c.V, c.E], BF16, kind="ExternalInput").ap()
    posT_t = dt_("posT", [c.E, c.TC], F32, kind="ExternalInput").ap()
    wqkv_t = dt_("wqkv", [c.L, c.E, 3 * c.H * c.HS], BF16,
                 kind="ExternalInput").ap()
    wo_t = dt_("wo", [c.L, c.E, c.E], BF16, kind="ExternalInput").ap()
    bo_t = dt_("bo", [c.L, c.E], F32, kind="ExternalInput").ap()
    ln1g_t = dt_("ln1g", [c.L, c.E], F32, kind="ExternalInput").ap()
    ln1b_t = dt_("ln1b", [c.L, c.E], F32, kind="ExternalInput").ap()
    ln2g_t = dt_("ln2g", [c.L, c.E], F32, kind="ExternalInput").ap()
    ln2b_t = dt_("ln2b", [c.L, c.E], F32, kind="ExternalInput").ap()
    w1_t = dt_("w1", [c.L, c.E, c.F], BF16, kind="ExternalInput").ap()
    b1_t = dt_("b1", [c.L, c.F], F32, kind="ExternalInput").ap()
    w2_t = dt_("w2", [c.L, c.F, c.E], BF16, kind="ExternalInput").ap()
    b2_t = dt_("b2", [c.L, c.E], F32, kind="ExternalInput").ap()
    lnfg_t = dt_("lnfg", [c.E], F32, kind="ExternalInput").ap()
    lnfb_t = dt_("lnfb", [c.E], F32, kind="ExternalInput").ap()
    wh_t = dt_("wh", [c.E, c.V], BF16, kind="ExternalInput").ap()
    mdiag_t = dt_("mdiag", [c.NTC, P, 2 * P], BF16,
                  kind="ExternalInput").ap()
    pidx_t = dt_("pidx", [P], I32, kind="ExternalInput").ap()
    oidx_t = dt_("oidx", [P], I32, kind="ExternalInput").ap()
    # logits written bf16; host upcasts to f32 and adds bh there
    out_t = dt_("out", [c.TC, c.V], BF16, kind="ExternalOutput").ap()

    with tile.TileContext(nc) as tc:
        with tc.tile_pool(name="sb", bufs=1) as sb, \
             tc.tile_pool(name="wpool", bufs=4) as wpool, \
             tc.tile_pool(name="xpool", bufs=2) as xpool, \
             tc.tile_pool(name="ps", bufs=4, space="PSUM") as ps, \
             tc.tile_pool(name="dram", bufs=1, space="DRAM") as dram:

            def psum(shape=None, dtype=F32, name="mm"):
                return ps.tile(shape or [P, 512], dtype, tag="mm", name=name)

            def psum4(name="mm4"):
                return ps.tile([P, 2 * 512], F32, tag="mm4", name=name,
                               bufs=2)

            # Projection phases need 4 [P,512] accumulators per output
            # group; alternating groups between the mm banks and the mm4
            # banks (idle outside attention) lets group g+1's matmuls run
            # while group g's psum->sbuf copies drain.
            pgc = [0]

            def psum_group(names):
                pgc[0] += 1
                if pgc[0] % 2 == 0:
                    return [psum(name=nm) for nm in names]
                a = psum4(name="pgA")
                b = psum4(name="pgB")
                views = [a[:, 0:512], a[:, 512:1024],
                         b[:, 0:512], b[:, 512:1024]]
                return views[:len(names)]

            # psum->sbuf drains alternate DVE/ACT to balance engine load
            cpc = [0]

            def drain_copy(out, in_):
                cpc[0] += 1
                if cpc[0] % 2 == 0:
                    nc.scalar.copy(out, in_)
                else:
                    nc.vector.tensor_copy(out=out, in_=in_)

            # ---- constants ----
            ones_bf = sb.tile([P, 1], BF16, tag="ones_bf", name="ones_bf")
            nc.vector.memset(ones_bf[:], 1.0)
            eps_sb = sb.tile([1, 1], F32, tag="eps", name="eps_sb")
            nc.vector.memset(eps_sb[:], 1e-5)
            ident = sb.tile([P, P], BF16, tag="ident", name="ident")
            make_identity(nc, ident[:])
            mdiag_sb = sb.tile([P, c.NTC, 2 * P], BF16, tag="mdiag",
                               name="mdiag_sb")
            for j in range(c.NTC):
                nc.sync.dma_start(mdiag_sb[:, j, :], mdiag_t[j])
            pidx_sb = sb.tile([P, 1], I32, tag="pidx", name="pidx_sb")
            nc.sync.dma_start(pidx_sb[:], pidx_t.rearrange("(p o) -> p o", o=1))
            oidx_sb = sb.tile([P, 1], I32, tag="oidx", name="oidx_sb")
            nc.sync.dma_start(oidx_sb[:], oidx_t.rearrange("(p o) -> p o", o=1))
            idx_sb = sb.tile([P, c.NTC], I32, tag="idx", name="idx_sb")
            nc.sync.dma_start(idx_sb[:], idx_t.rearrange("(tc p) -> p tc", p=P))

            # ---- collective staging: agi [2P, hrow]; own slot rows are
            # zeroed once so the pair ReduceScatter(add) yields exactly the
            # partner's hT in ago.
            agi_d = dram.tile([2 * P, c.hrow], BF16, tag="agi", name="agi")
            ago_d = dram.tile([P * c.hrow], BF16, tag="ago", name="ago")
            zT = sb.tile([P, c.hrow], BF16, tag="zT", name="zT")
            nc.vector.memset(zT[:], 0.0)
            nc.gpsimd.indirect_dma_start(
                out=agi_d[:], out_offset=bass.IndirectOffsetOnAxis(
                    ap=oidx_sb[:], axis=0),
                in_=zT[:], in_offset=None)

            for _rep in range(reps):
                # ---- residual stream xT[e, t] (f32), seeded with pos^T ----
                xT = sb.tile([P, c.NEC, c.TC], F32, tag="xT", name="xT")
                nc.sync.dma_start(
                    xT[:], posT_t.rearrange("(ec p) t -> p ec t", p=P))

                # ---- embedding gather + transpose ----
                for tcb in range(c.NTC):
                    emb = xpool.tile([P, c.E], BF16, tag="emb", name="emb")
                    if "gather" in ablate:
                        nc.sync.dma_start(emb[:],
                                          temb_t[tcb * P:(tcb + 1) * P, :])
                    else:
                        nc.gpsimd.indirect_dma_start(
                            out=emb[:], out_offset=None, in_=temb_t,
                            in_offset=bass.IndirectOffsetOnAxis(
                                ap=idx_sb[:, tcb:tcb + 1], axis=0))
                    for ec in range(c.NEC):
                        tps = psum([P, P], BF16, name="tps")
                        nc.tensor.transpose(
                            out=tps[:], in_=emb[:, ec * P:(ec + 1) * P],
                            identity=ident[:])
                        tpf = xpool.tile([P, P], F32, tag="tpf", name="tpf")
                        nc.vector.tensor_copy(out=tpf[:], in_=tps[:])
                        sl = xT[:, ec, tcb * P:(tcb + 1) * P]
                        nc.vector.tensor_tensor(out=sl, in0=sl, in1=tpf[:],
                                                op=OP.add)

                # ---- layernorm: xT -> out_bf (bf16 [P, NEC, TC]) ----
                # rstd = exp(-0.5*ln(var+eps)) stays in the exp/ln ACT set.
                def layernorm(xT, g_dram, b_dram, out_bf):
                    gb = sb.tile([P, 2 * c.NEC], F32, tag="gains", name="gb",
                                 bufs=2)
                    nc.sync.dma_start(
                        gb[:, 0:c.NEC], g_dram.rearrange("(ec p) -> p ec", p=P))
                    nc.sync.dma_start(
                        gb[:, c.NEC:], b_dram.rearrange("(ec p) -> p ec", p=P))
                    sum_ps = psum([1, c.TC], name="ln_sum")
                    sq_ps = psum([1, c.TC], name="ln_sq")
                    for ec in range(c.NEC):
                        xbf = xpool.tile([P, c.TC], BF16, tag="xbf", name="xbf")
                        nc.vector.tensor_copy(out=xbf[:], in_=xT[:, ec, :])
                        nc.tensor.matmul(out=sum_ps[:], lhsT=ones_bf[:],
                                         rhs=xbf[:], start=(ec == 0),
                                         stop=(ec == c.NEC - 1))
                        xsq = xpool.tile([P, c.TC], BF16, tag="xsq", name="xsq")
                        if ec % 2 == 0:
                            nc.scalar.activation(xsq[:], xbf[:], AF.Square)
                        else:
                            nc.vector.tensor_tensor(out=xsq[:], in0=xbf[:],
                                                    in1=xbf[:], op=OP.mult)
                        nc.tensor.matmul(out=sq_ps[:], lhsT=ones_bf[:],
                                         rhs=xsq[:], start=(ec == 0),
                                         stop=(ec == c.NEC - 1))
                    stats = xpool.tile([1, 3 * c.TC], F32, tag="stats",
                                       name="stats", bufs=1)
                    mean = stats[:, 0:c.TC]
                    var = stats[:, c.TC:2 * c.TC]
                    rstd = stats[:, 2 * c.TC:]
                    inv_e = 1.0 / c.E
                    # var+eps = (sq - sum^2/E)/E + eps with /E and eps folded
                    # into Ln's scale/bias; rstd = exp(-0.5*ln(var+eps)).
                    m2 = xpool.tile([1, c.TC], F32, tag="m2", name="m2")
                    nc.vector.tensor_scalar_mul(mean, sum_ps[:], inv_e)
                    nc.vector.tensor_tensor(out=m2[:], in0=mean, in1=mean,
                                            op=OP.mult)
                    nc.vector.tensor_scalar_mul(var, sq_ps[:], inv_e)
                    nc.vector.tensor_tensor(out=var, in0=var, in1=m2[:],
                                            op=OP.subtract)
                    nc.scalar.activation(var, var, AF.Ln, bias=eps_sb[:])
                    nc.scalar.activation(rstd, var, AF.Exp, scale=-0.5)
                    mrb = xpool.tile([P, 2, c.TC], F32, tag="mrb",
                                     name="mrb", bufs=1)
                    nc.gpsimd.partition_broadcast(mrb[:, 0, :], mean)
                    nc.gpsimd.partition_broadcast(mrb[:, 1, :], rstd)
                    for ec in range(c.NEC):
                        tmp = xpool.tile([P, c.TC], F32, tag="lntmp",
                                         name="lntmp")
                        nc.vector.tensor_tensor(out=tmp[:], in0=xT[:, ec, :],
                                                in1=mrb[:, 0, :],
                                                op=OP.subtract)
                        nc.vector.tensor_tensor(out=tmp[:], in0=tmp[:],
                                                in1=mrb[:, 1, :], op=OP.mult)
                        nc.vector.tensor_scalar(
                            out=out_bf[:, ec, :], in0=tmp[:],
                            scalar1=gb[:, ec:ec + 1],
                            scalar2=gb[:, c.NEC + ec:c.NEC + ec + 1],
                            op0=OP.mult, op1=OP.add)

                # qk-style projection: dst[feat_chunk, tok] from src hT-like
                def qk_proj(which, dst, src, l):
                    col0 = which * c.H * c.HS
                    for fcs in _chunks(range(c.NHP), 4):
                        pss = dict(zip(fcs, psum_group(
                            [f"qk{fc}" for fc in fcs])))
                        w = len(fcs) * P
                        wt = wpool.tile([P, c.NEC, w], BF16,
                                        tag="wblk", name="wt")
                        nc.sync.dma_start(
                            wt[:],
                            wqkv_t[l, :, col0 + fcs[0] * P:
                                   col0 + fcs[0] * P + w]
                            .rearrange("(ec p) w -> p ec w", p=P))
                        for ec in range(c.NEC):
                            for j, fc in enumerate(fcs):
                                nc.tensor.matmul(
                                    out=pss[fc][:, :c.TC],
                                    lhsT=wt[:, ec, j * P:(j + 1) * P],
                                    rhs=src[:, ec, :],
                                    start=(ec == 0),
                                    stop=(ec == c.NEC - 1))
                        for fc in fcs:
                            drain_copy(dst[:, fc, :], pss[fc][:, :c.TC])

                # v projection (natural [tok, vrow] layout, ones col per head)
                def v_proj(dst, src, l):
                    vw = min(512, c.H * c.HS)
                    nvh = (c.H * c.HS) // vw
                    hs_per_vh = vw // c.HS
                    col0 = 2 * c.H * c.HS
                    vjobs = [(tcb, vh) for tcb in range(c.NTC)
                             for vh in range(nvh)]
                    for grp in _chunks(vjobs, 4):
                        pss = dict(zip(grp, psum_group(
                            [f"v{j[0]}_{j[1]}" for j in grp])))
                        wts = {}
                        for vh in sorted({vh for _, vh in grp}):
                            wt = wpool.tile([P, c.NEC, vw], BF16,
                                            tag="wblk", name="wt")
                            nc.sync.dma_start(
                                wt[:],
                                wqkv_t[l, :, col0 + vh * vw:
                                       col0 + (vh + 1) * vw]
                                .rearrange("(ec p) w -> p ec w", p=P))
                            wts[vh] = wt
                        for ec in range(c.NEC):
                            for tcb, vh in grp:
                                nc.tensor.matmul(
                                    out=pss[(tcb, vh)][:, :vw],
                                    lhsT=src[:, ec, tcb * P:(tcb + 1) * P],
                                    rhs=wts[vh][:, ec, :],
                                    start=(ec == 0), stop=(ec == c.NEC - 1))
                        for tcb, vh in grp:
                            for hh in range(hs_per_vh):
                                h = vh * hs_per_vh + hh
                                drain_copy(
                                    dst[:, tcb, h * c.HP:h * c.HP + c.HS],
                                    pss[(tcb, vh)][:, hh * c.HS:
                                                   (hh + 1) * c.HS])

                # ============ layers ============
                for l in range(c.L):
                    hT = sb.tile([P, c.NEC, c.TC], BF16, tag="hT", name="hT")
                    layernorm(xT, ln1g_t[l], ln1b_t[l], hT)

                    # ---- pair exchange of hT via zero-slot ReduceScatter ----
                    if "coll" not in ablate:
                        nc.gpsimd.indirect_dma_start(
                            out=agi_d[:],
                            out_offset=bass.IndirectOffsetOnAxis(
                                ap=pidx_sb[:], axis=0),
                            in_=hT[:].rearrange("p a b -> p (a b)"),
                            in_offset=None)
                        nc.gpsimd.collective_compute(
                            "ReduceScatter", OP.add, replica_groups=PAIRS,
                            ins=[agi_d[:].rearrange("a b -> (a b)")],
                            outs=[ago_d[:]])
                    else:
                        nc.sync.dma_start(
                            ago_d[:].rearrange("(p w) -> p w", p=P),
                            hT[:].rearrange("p a b -> p (a b)"))

                    # ---- own-half projections overlap the collective ----
                    kf_own = sb.tile([P, c.NHP, c.TC], BF16, tag="kfo",
                                     name="kf_own")
                    qT = sb.tile([P, c.NHP, c.TC], BF16, tag="qT", name="qT")
                    vf_own = sb.tile([P, c.NTC, c.vrow], BF16, tag="vfo",
                                     name="vf_own")
                    vf_par = sb.tile([P, c.NTC, c.vrow], BF16, tag="vfp",
                                     name="vf_par")
                    for h in range(c.H):
                        for v_ in (vf_own, vf_par):
                            nc.vector.memset(
                                v_[:, :, h * c.HP + c.HS:
                                   h * c.HP + c.HS + 1], 1.0)
                    if "qkv" in ablate:
                        nc.vector.memset(qT[:], 0.0078125)
                        nc.vector.memset(kf_own[:], 0.0078125)
                    else:
                        qk_proj(1, kf_own, hT, l)
                        qk_proj(0, qT, hT, l)
                        v_proj(vf_own, hT, l)

                    # ---- partner h arrives; recompute partner k/v ----
                    # hf load goes through gpsimd (SWDGE): its wait on the
                    # collective must not head-of-line-block the sync DGE
                    # queue, where it would stall the partner weight
                    # prefetches that have no ago dependency.
                    hf = sb.tile([P, c.NEC, c.TC], BF16, tag="hfp",
                                 name="hfp")
                    nc.gpsimd.dma_start(
                        hf[:], ago_d[:].rearrange("(p ec t) -> p ec t",
                                                  p=P, ec=c.NEC))
                    kf_par = sb.tile([P, c.NHP, c.TC], BF16, tag="kfp",
                                     name="kf_par")
                    if "qkv" in ablate:
                        nc.vector.memset(kf_par[:], 0.0078125)
                    else:
                        qk_proj(1, kf_par, hf, l)
                        v_proj(vf_par, hf, l)

                    # ---- attention ----
                    # q chunk j attends own chunks i<=j (diag: tril) and
                    # partner chunks i<=j (diag: msk01 0/1 per core).
                    attT = sb.tile([P, c.NHP, c.TC], BF16, tag="attT",
                                   name="attT")
                    if "attn" in ablate:
                        nc.vector.memset(attT[:], 0.00390625)
                    for h in range(0 if "attn" in ablate else c.H):
                        hp, hb = divmod(h, c.HPP)
                        p0 = hb * c.HS
                        att_ps = psum(name="att_ps")
                        for grp in ([0, 1], [2], [3]):
                            offs, o = {}, 0
                            for j in grp:
                                offs[j] = o
                                o += 2 * (j + 1) * P
                            s4 = psum4(name="s4")
                            # block order per j: own non-diag, own diag,
                            # par diag, par non-diag -- the two diag blocks
                            # are adjacent so one [P,2P] multiply masks both
                            for j in grp:
                                q0 = j * P
                                blks = ([(kf_own, i) for i in range(j)]
                                        + [(kf_own, j), (kf_par, j)]
                                        + [(kf_par, i) for i in range(j)])
                                for bi, (kfx, i) in enumerate(blks):
                                    col = offs[j] + bi * P
                                    nc.tensor.matmul(
                                        out=s4[:, col:col + P],
                                        lhsT=kfx[p0:p0 + c.HS, hp,
                                                 i * P:(i + 1) * P],
                                        rhs=qT[p0:p0 + c.HS, hp,
                                               q0:q0 + P],
                                        start=True, stop=True)
                            ex = xpool.tile([P, 8 * P], BF16, tag="ex",
                                            name="ex", bufs=3)
                            if "exp" in ablate:
                                nc.vector.tensor_copy(
                                    out=ex[:, :o], in_=s4[:, :o])
                            else:
                                nc.scalar.activation(
                                    ex[:, :o], s4[:, :o],
                                    AF.Exp, scale=c.scale)
                                for j in grp:
                                    od = offs[j] + (j - 1) * P + P
                                    nc.vector.tensor_tensor(
                                        out=ex[:, od:od + 2 * P],
                                        in0=ex[:, od:od + 2 * P],
                                        in1=mdiag_sb[:, j, :], op=OP.mult)
                            for j in grp:
                                q0 = j * P
                                nblk = 2 * (j + 1)
                                if "av" in ablate:
                                    nc.vector.memset(
                                        att_ps[:c.HP, q0:q0 + P], 0.0078125)
                                    continue
                                vblks = ([(vf_own, i) for i in range(j)]
                                         + [(vf_own, j), (vf_par, j)]
                                         + [(vf_par, i) for i in range(j)])
                                for bi, (vfx, i) in enumerate(vblks):
                                    col = offs[j] + bi * P
                                    nc.tensor.matmul(
                                        out=att_ps[:c.HP, q0:q0 + P],
                                        lhsT=vfx[:, i,
                                                 h * c.HP:(h + 1) * c.HP],
                                        rhs=ex[:, col:col + P],
                                        start=(bi == 0),
                                        stop=(bi == nblk - 1))
                        rec = xpool.tile([1, c.TC], F32, tag="rec",
                                         name="rec", bufs=2)
                        nc.vector.reciprocal(rec[:], att_ps[c.HS:c.HP, :c.TC])
                        recb = xpool.tile([c.HS, c.TC], F32, tag="recb",
                                          name="recb", bufs=2)
                        nc.gpsimd.partition_broadcast(recb[:], rec[:])
                        nc.vector.tensor_tensor(
                            out=attT[p0:p0 + c.HS, hp, :],
                            in0=att_ps[:c.HS, :c.TC],
                            in1=recb[:], op=OP.mult)

                    # ---- Wo projection + bo + residual ----
                    bob = sb.tile([P, c.NEC], F32, tag="bob", name="bob",
                                  bufs=2)
                    nc.sync.dma_start(
                        bob[:], bo_t[l].rearrange("(ec p) -> p ec", p=P))
                    for eos in _chunks(range(c.NEC), 4):
                        pss = dict(zip(eos, psum_group(
                            [f"wo{eo}" for eo in eos])))
                        w = len(eos) * P
                        wt = wpool.tile([P, c.NEC, w], BF16,
                                        tag="wblk", name="wt")
                        nc.sync.dma_start(
                            wt[:], wo_t[l, :, eos[0] * P:eos[0] * P + w]
                            .rearrange("(ec p) w -> p ec w", p=P))
                        for ec in range(c.NEC):
                            for j, eo in enumerate(eos):
                                nc.tensor.matmul(
                                    out=pss[eo][:, :c.TC],
                                    lhsT=wt[:, ec, j * P:(j + 1) * P],
                                    rhs=attT[:, ec, :],
                                    start=(ec == 0), stop=(ec == c.NEC - 1))
                        for eo in eos:
                            nc.vector.scalar_tensor_tensor(
                                out=xT[:, eo, :], in0=pss[eo][:, :c.TC],
                                scalar=bob[:, eo:eo + 1],
                                in1=xT[:, eo, :], op0=OP.add, op1=OP.add)

                    # ---- LN2 + FFN ----
                    h2T = sb.tile([P, c.NEC, c.TC], BF16, tag="hT", name="h2T")
                    layernorm(xT, ln2g_t[l], ln2b_t[l], h2T)

                    b1b = sb.tile([P, c.NFC], F32, tag="b1b", name="b1b",
                                  bufs=2)
                    nc.sync.dma_start(
                        b1b[:], b1_t[l].rearrange("(fc p) -> p fc", p=P))
                    uT = sb.tile([P, c.NFC, c.TC], BF16, tag="uT", name="uT")
                    if "ffn" in ablate:
                        nc.vector.memset(uT[:], 0.0078125)
                    for fcs in ([] if "ffn" in ablate
                                else _chunks(range(c.NFC), 4)):
                        pss = dict(zip(fcs, psum_group(
                            [f"u{fc}" for fc in fcs])))
                        w = len(fcs) * P
                        wt = wpool.tile([P, c.NEC, w], BF16,
                                        tag="wblk", name="wt")
                        nc.sync.dma_start(
                            wt[:], w1_t[l, :, fcs[0] * P:fcs[0] * P + w]
                            .rearrange("(ec p) w -> p ec w", p=P))
                        for ec in range(c.NEC):
                            for j, fc in enumerate(fcs):
                                nc.tensor.matmul(
                                    out=pss[fc][:, :c.TC],
                                    lhsT=wt[:, ec, j * P:(j + 1) * P],
                                    rhs=h2T[:, ec, :],
                                    start=(ec == 0), stop=(ec == c.NEC - 1))
                        for fc in fcs:
                            # relu(x + b1): alternate ACT / DVE
                            if fc % 2 == 0:
                                nc.scalar.activation(
                                    uT[:, fc, :], pss[fc][:, :c.TC],
                                    AF.Relu, bias=b1b[:, fc:fc + 1])
                            else:
                                nc.vector.tensor_scalar(
                                    out=uT[:, fc, :], in0=pss[fc][:, :c.TC],
                                    scalar1=b1b[:, fc:fc + 1], scalar2=0.0,
                                    op0=OP.add, op1=OP.max)

                    b2b = sb.tile([P, c.NEC], F32, tag="bob", name="b2b",
                                  bufs=2)
                    nc.sync.dma_start(
                        b2b[:], b2_t[l].rearrange("(ec p) -> p ec", p=P))
                    for eos in ([] if "ffn" in ablate
                                else _chunks(range(c.NEC), 4)):
                        pss = dict(zip(eos, psum_group(
                            [f"y{eo}" for eo in eos])))
                        w = len(eos) * P
                        for kcs in _chunks(range(c.NFC), 8):
                            wt = wpool.tile([P, len(kcs), w], BF16,
                                            tag="wblk", name="wt")
                            nc.sync.dma_start(
                                wt[:], w2_t[l, kcs[0] * P:
                                            (kcs[-1] + 1) * P,
                                            eos[0] * P:eos[0] * P + w]
                                .rearrange("(kc p) w -> p kc w", p=P))
                            for ki, kc in enumerate(kcs):
                                for j, eo in enumerate(eos):
                                    nc.tensor.matmul(
                                        out=pss[eo][:, :c.TC],
                                        lhsT=wt[:, ki, j * P:(j + 1) * P],
                                        rhs=uT[:, kc, :],
                                        start=(kc == 0),
                                        stop=(kc == c.NFC - 1))
                        for eo in eos:
                            nc.vector.scalar_tensor_tensor(
                                out=xT[:, eo, :], in0=pss[eo][:, :c.TC],
                                scalar=b2b[:, eo:eo + 1],
                                in1=xT[:, eo, :], op0=OP.add, op1=OP.add)

                # ============ final LN + lm_head ============
                xlnT = sb.tile([P, c.NEC, c.TC], BF16, tag="hT", name="xlnT")
                layernorm(xT, lnfg_t, lnfb_t, xlnT)

                vcs = [] if "lmhead" in ablate else c.vchunks
                for v0, wv in vcs:
                    pss = dict(zip(range(c.NTC), psum_group(
                        [f"lg{t}" for t in range(c.NTC)])))
                    wt = wpool.tile([P, c.NEC, 512], BF16, tag="wblk",
                                    name="wt")
                    nc.sync.dma_start(
                        wt[:, :, :wv], wh_t[:, v0:v0 + wv]
                        .rearrange("(ec p) w -> p ec w", p=P))
                    for ec in range(c.NEC):
                        for tcb in range(c.NTC):
                            nc.tensor.matmul(
                                out=pss[tcb][:, :wv],
                                lhsT=xlnT[:, ec, tcb * P:(tcb + 1) * P],
                                rhs=wt[:, ec, :wv],
                                start=(ec == 0), stop=(ec == c.NEC - 1))
                    lg = xpool.tile([P, c.NTC, 512], BF16, tag="lg",
                                    name="lg", bufs=2)
                    for tcb in range(c.NTC):
                        drain_copy(lg[:, tcb, :wv], pss[tcb][:, :wv])
                    nc.sync.dma_start(
                        out_t[:, v0:v0 + wv]
                        .rearrange("(tcb p) w -> p tcb w", p=P),
                        lg[:, :, :wv])

    nc.compile()
    return nc


# ----------------------------------------------------------------------------
# host side
# ----------------------------------------------------------------------------

def prep_inputs(c: Cfg, inputs):
    """Build the 8 per-core input maps from the full model inputs."""
    bf = ml_dtypes.bfloat16
    f32 = np.float32

    idx = np.asarray(inputs["idx"]).astype(np.int32)
    temb = np.asarray(inputs["tok_emb"], f32).astype(bf)
    pos = np.asarray(inputs["pos_emb"], f32)
    Wq, Wk, Wv = (np.asarray(inputs[k], f32) for k in ("Wq", "Wk", "Wv"))
    EHH = c.H * c.HS
    wqkv = np.ascontiguousarray(np.concatenate(
        [w.transpose(0, 2, 1, 3).reshape(c.L, c.E, EHH)
         for w in (Wq, Wk, Wv)], axis=2).astype(bf))

    kk = np.arange(P)[:, None]
    qq = np.arange(P)[None, :]
    tril = (kk <= qq).astype(np.float32)

    shared = {
        "temb": temb, "wqkv": wqkv,
        "wo": np.asarray(inputs["Wo"], f32).astype(bf),
        "w1": np.asarray(inputs["W1"], f32).astype(bf),
        "w2": np.asarray(inputs["W2"], f32).astype(bf),
        "wh": np.asarray(inputs["Wh"], f32).astype(bf),
        "bo": np.asarray(inputs["bo"], f32),
        "ln1g": np.asarray(inputs["ln1_g"], f32),
        "ln1b": np.asarray(inputs["ln1_b"], f32),
        "ln2g": np.asarray(inputs["ln2_g"], f32),
        "ln2b": np.asarray(inputs["ln2_b"], f32),
        "b1": np.asarray(inputs["b1"], f32),
        "b2": np.asarray(inputs["b2"], f32),
        "lnfg": np.asarray(inputs["lnf_g"], f32),
        "lnfb": np.asarray(inputs["lnf_b"], f32),
    }

    in_maps = []
    for core in range(N_CORES):
        b, half = divmod(core, 2)
        own = c.CHUNKS[half]
        par = c.CHUNKS[1 - half]
        tok = np.concatenate([np.arange(g * P, (g + 1) * P) for g in own])
        # per q chunk j: [tril | partner-diag 0/1] masking the two
        # adjacent diagonal blocks of the score layout
        mdiag = np.zeros((c.NTC, P, 2 * P), np.float32)
        for j in range(c.NTC):
            mdiag[j, :, 0:P] = tril
            mdiag[j, :, P:] = 1.0 if par[j] < own[j] else 0.0
        in_maps.append(dict(
            shared,
            idx=np.ascontiguousarray(idx[b, tok]),
            posT=np.ascontiguousarray(pos[tok].T),
            mdiag=np.ascontiguousarray(mdiag.astype(bf)),
            pidx=((1 - half) * P + np.arange(P)).astype(np.int32),
            oidx=(half * P + np.arange(P)).astype(np.int32),
        ))
    return in_maps


_CACHE = {}


def _get_program():
    if "nc" not in _CACHE:
        _CACHE["cfg"] = Cfg()
        _CACHE["nc"] = build_program(_CACHE["cfg"])
    return _CACHE["nc"], _CACHE["cfg"]


def kernel(**inputs) -> np.ndarray:
    nc, c = _get_program()
    in_maps = prep_inputs(c, inputs)
    res = bass_utils.run_bass_kernel_spmd(
        nc, in_maps, core_ids=list(range(N_CORES)))
    out = np.empty((c.B, c.T, c.V), np.float32)
    for core in range(N_CORES):
        b, half = divmod(core, 2)
        o = res.results[core]["out"].astype(np.float32)
        for i, g in enumerate(c.CHUNKS[half]):
            out[b, g * P:(g + 1) * P] = o[i * P:(i + 1) * P]
    out += np.asarray(inputs["bh"], np.float32)
    return out


# revision 34
# speedup vs baseline: 1.3089x; 1.0195x over previous
"""Trainium2 Bass kernel for a 4-layer decoder transformer (B4 T1024 E1024 H16
hs64 F4096 V32000) on 8 NeuronCores.

Sharding: batch(4) x sequence-half(2). Core c handles batch b=c//2 and the
causal-interleaved token chunks CHUNKS[c%2]. The residual stream lives in
SBUF transposed (xT: [E, 512], E on partitions) so every matmul has its
contraction dim on partitions.

Cross-core exchange (v2): instead of AllGathering k|v (2MB in, 4MB out per
core), each core sends only its LN1 output hT (1MB) to its pair partner and
recomputes the partner's k/v locally. The exchange is a pair
ReduceScatter(add): each core scatters hT into the PARTNER's slot of agi
(own slot pre-zeroed once), so RS yields exactly the partner's hT -- half
the collective bytes of an AllGather of the same payload, and the
per-core-different slot offset is handled by an indirect DMA driven by a
host-supplied row-index input (keeps the program SPMD-uniform).

Attention indexes keys as (own/partner, chunk) triangular blocks: q chunk j
attends own chunks i<=j (diagonal tril-masked) and partner chunks i<=j
(diagonal 0/1-masked per core, input msk01). This is uniform across cores
and does the same 20 score blocks per head as the previous POSF scheme.

PSUM layout: tag "mm" = 4 rotating single-bank tiles, tag "mm4" = [128,
1024] 2-bank tiles (bufs=2) used for score batches so exp runs as 3 big ACT
ops per head. All matmul phases use 4-bank accumulation blocks so
consecutive blocks double-buffer. Elementwise psum drains alternate between
DVE and ACT to balance engine load. LN rstd = exp(-0.5*ln(var+eps)) keeps
every ACT call in the natural_log_exp table set (no table reloads).
"""

import numpy as np
import ml_dtypes

import concourse.bass as bass
import concourse.bacc as bacc
import concourse.mybir as mybir
import concourse.tile as tile
from concourse import bass_utils
from concourse.masks import make_identity

F32 = mybir.dt.float32
F32R = mybir.dt.float32r
BF16 = mybir.dt.bfloat16
I32 = mybir.dt.int32
AF = mybir.ActivationFunctionType
OP = mybir.AluOpType
P = 128

N_CORES = 8
PAIRS = [[0, 1], [2, 3], [4, 5], [6, 7]]


def _chunks(seq, n):
    seq = list(seq)
    return [seq[i:i + n] for i in range(0, len(seq), n)]


class Cfg:
    def __init__(self, B=4, T=1024, E=1024, H=16, HS=64, L=4, F=4096, V=32000):
        self.B, self.T, self.E, self.H, self.HS = B, T, E, H, HS
        self.L, self.F, self.V = L, F, V
        self.TC = T // 2                    # tokens per core
        self.NEC = E // P                   # E chunks (partition tiles)
        self.NTC = self.TC // P             # local token chunks
        self.NFC = F // P                   # FFN hidden chunks
        self.HPP = P // HS                  # heads per 128-partition tile
        self.NHP = (H * HS) // P            # head-pair tiles
        self.HP = HS + 1                    # augmented per-head stride in v
        self.scale = 1.0 / (E ** 0.5)
        self.hrow = self.NEC * self.TC      # per-partition hT row (4096)
        self.vchunks = []
        v0 = 0
        while v0 < V:
            self.vchunks.append((v0, min(512, V - v0)))
            v0 += 512
        self.vrow = H * self.HP
        # Causal-interleaved token chunks: core half h owns global 128-token
        # chunks CHUNKS[h] (in this local order). Both lists ascend, so
        # own-vs-own causality is exactly i<=j with tril on the diagonal,
        # and partner chunk i is valid for q chunk j iff
        # CHUNKS[partner][i] < CHUNKS[own][j] -- which holds for all i<j and
        # alternates on the diagonal (host input msk01).
        self.CHUNKS = [[0, 3, 4, 7], [1, 2, 5, 6]]


def build_program(c: Cfg, reps: int = 1, ablate=()):
    # Every ACT call here uses only {Exp, Ln, Relu, Copy, Identity}, all of
    # which live in the natural_log_exp_and_others table set. The stock
    # table chooser maps Exp->exp_and_others and Ln->natural_log, inserting
    # a ~1.3us table load per switch (2 per layernorm). Restrict the
    # eligible sets (ids preserved) during this build so one load serves
    # the whole program.
    import concourse.bacc as _bacc_mod
    _orig_tables = _bacc_mod.get_activation_tables

    def _only_ln_exp(arch):
        tabs = _orig_tables(arch)
        return {name: (s if name == "natural_log_exp_and_others" else set())
                for name, s in tabs.items()}

    _bacc_mod.get_activation_tables = _only_ln_exp
    try:
        return _build_program_inner(c, reps, ablate)
    finally:
        _bacc_mod.get_activation_tables = _orig_tables


def _build_program_inner(c: Cfg, reps: int = 1, ablate=()):
    nc = bacc.Bacc("TRN2", target_bir_lowering=False, debug=False,
                   num_devices=N_CORES)

    # ---- DRAM I/O ----
    dt_ = nc.dram_tensor
    idx_t = dt_("idx", [c.TC], I32, kind="ExternalInput").ap()
    temb_t = dt_("temb", [c.V, c.E], BF16, kind="ExternalInput").ap()
    posT_t = dt_("posT", [c.E, c.TC], F32, kind="ExternalInput").ap()
    wqkv_t = dt_("wqkv", [c.L, c.E, 3 * c.H * c.HS], BF16,
                 kind="ExternalInput").ap()
    wo_t = dt_("wo", [c.L, c.E, c.E], BF16, kind="ExternalInput").ap()
    bo_t = dt_("bo", [c.L, c.E], F32, kind="ExternalInput").ap()
    ln1g_t = dt_("ln1g", [c.L, c.E], F32, kind="ExternalInput").ap()
    ln1b_t = dt_("ln1b", [c.L, c.E], F32, kind="ExternalInput").ap()
    ln2g_t = dt_("ln2g", [c.L, c.E], F32, kind="ExternalInput").ap()
    ln2b_t = dt_("ln2b", [c.L, c.E], F32, kind="ExternalInput").ap()
    w1_t = dt_("w1", [c.L, c.E, c.F], BF16, kind="ExternalInput").ap()
    b1_t = dt_("b1", [c.L, c.F], F32, kind="ExternalInput").ap()
    w2_t = dt_("w2", [c.L, c.F, c.E], BF16, kind="ExternalInput").ap()
    b2_t = dt_("b2", [c.L, c.E], F32, kind="ExternalInput").ap()
    lnfg_t = dt_("lnfg", [c.E], F32, kind="ExternalInput").ap()
    lnfb_t = dt_("lnfb", [c.E], F32, kind="ExternalInput").ap()
    wh_t = dt_("wh", [c.E, c.V], BF16, kind="ExternalInput").ap()
    mdiag_t = dt_("mdiag", [c.NTC, P, 2 * P], BF16,
                  kind="ExternalInput").ap()
    pidx_t = dt_("pidx", [P], I32, kind="ExternalInput").ap()
    oidx_t = dt_("oidx", [P], I32, kind="ExternalInput").ap()
    # logits written bf16; host upcasts to f32 and adds bh there
    out_t = dt_("out", [c.TC, c.V], BF16, kind="ExternalOutput").ap()

    with tile.TileContext(nc) as tc:
        with tc.tile_pool(name="sb", bufs=1) as sb, \
             tc.tile_pool(name="wpool", bufs=4) as wpool, \
             tc.tile_pool(name="xpool", bufs=2) as xpool, \
             tc.tile_pool(name="ps", bufs=4, space="PSUM") as ps, \
             tc.tile_pool(name="dram", bufs=1, space="DRAM") as dram:

            def psum(shape=None, dtype=F32, name="mm"):
                return ps.tile(shape or [P, 512], dtype, tag="mm", name=name)

            def psum4(name="mm4"):
                return ps.tile([P, 2 * 512], F32, tag="mm4", name=name,
                               bufs=2)

            # Projection phases need 4 [P,512] accumulators per output
            # group; alternating groups between the mm banks and the mm4
            # banks (idle outside attention) lets group g+1's matmuls run
            # while group g's psum->sbuf copies drain.
            pgc = [0]

            def psum_group(names):
                pgc[0] += 1
                if pgc[0] % 2 == 0:
                    return [psum(name=nm) for nm in names]
                a = psum4(name="pgA")
                b = psum4(name="pgB")
                views = [a[:, 0:512], a[:, 512:1024],
                         b[:, 0:512], b[:, 512:1024]]
                return views[:len(names)]

            # psum->sbuf drains alternate DVE/ACT to balance engine load
            cpc = [0]

            def drain_copy(out, in_):
                cpc[0] += 1
                if cpc[0] % 2 == 0:
                    nc.scalar.copy(out, in_)
                else:
                    nc.vector.tensor_copy(out=out, in_=in_)

            # ---- constants ----
            ones_bf = sb.tile([P, 1], BF16, tag="ones_bf", name="ones_bf")
            nc.vector.memset(ones_bf[:], 1.0)
            eps_sb = sb.tile([1, 1], F32, tag="eps", name="eps_sb")
            nc.vector.memset(eps_sb[:], 1e-5)
            ident = sb.tile([P, P], BF16, tag="ident", name="ident")
            make_identity(nc, ident[:])
            mdiag_sb = sb.tile([P, c.NTC, 2 * P], BF16, tag="mdiag",
                               name="mdiag_sb")
            for j in range(c.NTC):
                nc.sync.dma_start(mdiag_sb[:, j, :], mdiag_t[j])
            pidx_sb = sb.tile([P, 1], I32, tag="pidx", name="pidx_sb")
            nc.sync.dma_start(pidx_sb[:], pidx_t.rearrange("(p o) -> p o", o=1))
            oidx_sb = sb.tile([P, 1], I32, tag="oidx", name="oidx_sb")
            nc.sync.dma_start(oidx_sb[:], oidx_t.rearrange("(p o) -> p o", o=1))
            idx_sb = sb.tile([P, c.NTC], I32, tag="idx", name="idx_sb")
            nc.sync.dma_start(idx_sb[:], idx_t.rearrange("(tc p) -> p tc", p=P))

            # ---- collective staging: agi [2P, hrow]; own slot rows are
            # zeroed once so the pair ReduceScatter(add) yields exactly the
            # partner's hT in ago.
            agi_d = dram.tile([2 * P, c.hrow], BF16, tag="agi", name="agi")
            ago_d = dram.tile([P * c.hrow], BF16, tag="ago", name="ago")
            zT = sb.tile([P, c.hrow], BF16, tag="zT", name="zT")
            nc.vector.memset(zT[:], 0.0)
            nc.gpsimd.indirect_dma_start(
                out=agi_d[:], out_offset=bass.IndirectOffsetOnAxis(
                    ap=oidx_sb[:], axis=0),
                in_=zT[:], in_offset=None)

            for _rep in range(reps):
                # ---- residual stream xT[e, t] (f32), seeded with pos^T ----
                xT = sb.tile([P, c.NEC, c.TC], F32, tag="xT", name="xT")
                nc.sync.dma_start(
                    xT[:], posT_t.rearrange("(ec p) t -> p ec t", p=P))

                # ---- embedding gather + transpose ----
                for tcb in range(c.NTC):
                    emb = xpool.tile([P, c.E], BF16, tag="emb", name="emb")
                    if "gather" in ablate:
                        nc.sync.dma_start(emb[:],
                                          temb_t[tcb * P:(tcb + 1) * P, :])
                    else:
                        nc.gpsimd.indirect_dma_start(
                            out=emb[:], out_offset=None, in_=temb_t,
                            in_offset=bass.IndirectOffsetOnAxis(
                                ap=idx_sb[:, tcb:tcb + 1], axis=0))
                    for ec in range(c.NEC):
                        tps = psum([P, P], BF16, name="tps")
                        nc.tensor.transpose(
                            out=tps[:], in_=emb[:, ec * P:(ec + 1) * P],
                            identity=ident[:])
                        tpf = xpool.tile([P, P], F32, tag="tpf", name="tpf")
                        nc.vector.tensor_copy(out=tpf[:], in_=tps[:])
                        sl = xT[:, ec, tcb * P:(tcb + 1) * P]
                        nc.vector.tensor_tensor(out=sl, in0=sl, in1=tpf[:],
                                                op=OP.add)

                # ---- layernorm: xT -> out_bf (bf16 [P, NEC, TC]) ----
                # rstd = exp(-0.5*ln(var+eps)) stays in the exp/ln ACT set.
                def layernorm(xT, g_dram, b_dram, out_bf):
                    gb = sb.tile([P, 2 * c.NEC], F32, tag="gains", name="gb",
                                 bufs=2)
                    nc.sync.dma_start(
                        gb[:, 0:c.NEC], g_dram.rearrange("(ec p) -> p ec", p=P))
                    nc.sync.dma_start(
                        gb[:, c.NEC:], b_dram.rearrange("(ec p) -> p ec", p=P))
                    sum_ps = psum([1, c.TC], name="ln_sum")
                    sq_ps = psum([1, c.TC], name="ln_sq")
                    for ec in range(c.NEC):
                        xbf = xpool.tile([P, c.TC], BF16, tag="xbf", name="xbf")
                        nc.vector.tensor_copy(out=xbf[:], in_=xT[:, ec, :])
                        nc.tensor.matmul(out=sum_ps[:], lhsT=ones_bf[:],
                                         rhs=xbf[:], start=(ec == 0),
                                         stop=(ec == c.NEC - 1))
                        xsq = xpool.tile([P, c.TC], BF16, tag="xsq", name="xsq")
                        if ec % 2 == 0:
                            nc.scalar.activation(xsq[:], xbf[:], AF.Square)
                        else:
                            nc.vector.tensor_tensor(out=xsq[:], in0=xbf[:],
                                                    in1=xbf[:], op=OP.mult)
                        nc.tensor.matmul(out=sq_ps[:], lhsT=ones_bf[:],
                                         rhs=xsq[:], start=(ec == 0),
                                         stop=(ec == c.NEC - 1))
                    stats = xpool.tile([1, 3 * c.TC], F32, tag="stats",
                                       name="stats", bufs=1)
                    mean = stats[:, 0:c.TC]
                    var = stats[:, c.TC:2 * c.TC]
                    rstd = stats[:, 2 * c.TC:]
                    inv_e = 1.0 / c.E
                    # var+eps = (sq - sum^2/E)/E + eps with /E and eps folded
                    # into Ln's scale/bias; rstd = exp(-0.5*ln(var+eps)).
                    m2 = xpool.tile([1, c.TC], F32, tag="m2", name="m2")
                    nc.vector.tensor_scalar_mul(mean, sum_ps[:], inv_e)
                    nc.vector.tensor_tensor(out=m2[:], in0=mean, in1=mean,
                                            op=OP.mult)
                    nc.vector.tensor_scalar_mul(var, sq_ps[:], inv_e)
                    nc.vector.tensor_tensor(out=var, in0=var, in1=m2[:],
                                            op=OP.subtract)
                    nc.scalar.activation(var, var, AF.Ln, bias=eps_sb[:])
                    nc.scalar.activation(rstd, var, AF.Exp, scale=-0.5)
                    mrb = xpool.tile([P, 2, c.TC], F32, tag="mrb",
                                     name="mrb", bufs=1)
                    nc.gpsimd.partition_broadcast(mrb[:, 0, :], mean)
                    nc.gpsimd.partition_broadcast(mrb[:, 1, :], rstd)
                    for ec in range(c.NEC):
                        tmp = xpool.tile([P, c.TC], F32, tag="lntmp",
                                         name="lntmp")
                        nc.vector.tensor_tensor(out=tmp[:], in0=xT[:, ec, :],
                                                in1=mrb[:, 0, :],
                                                op=OP.subtract)
                        nc.vector.tensor_tensor(out=tmp[:], in0=tmp[:],
                                                in1=mrb[:, 1, :], op=OP.mult)
                        nc.vector.tensor_scalar(
                            out=out_bf[:, ec, :], in0=tmp[:],
                            scalar1=gb[:, ec:ec + 1],
                            scalar2=gb[:, c.NEC + ec:c.NEC + ec + 1],
                            op0=OP.mult, op1=OP.add)

                # qk-style projection: dst[feat_chunk, tok] from src hT-like
                def qk_proj(which, dst, src, l):
                    col0 = which * c.H * c.HS
                    for fcs in _chunks(range(c.NHP), 4):
                        pss = dict(zip(fcs, psum_group(
                            [f"qk{fc}" for fc in fcs])))
                        w = len(fcs) * P
                        wt = wpool.tile([P, c.NEC, w], BF16,
                                        tag="wblk", name="wt")
                        nc.sync.dma_start(
                            wt[:],
                            wqkv_t[l, :, col0 + fcs[0] * P:
                                   col0 + fcs[0] * P + w]
                            .rearrange("(ec p) w -> p ec w", p=P))
                        for ec in range(c.NEC):
                            for j, fc in enumerate(fcs):
                                nc.tensor.matmul(
                                    out=pss[fc][:, :c.TC],
                                    lhsT=wt[:, ec, j * P:(j + 1) * P],
                                    rhs=src[:, ec, :],
                                    start=(ec == 0),
                                    stop=(ec == c.NEC - 1))
                        for fc in fcs:
                            drain_copy(dst[:, fc, :], pss[fc][:, :c.TC])

                # v projection (natural [tok, vrow] layout, ones col per head)
                def v_proj(dst, src, l):
                    vw = min(512, c.H * c.HS)
                    nvh = (c.H * c.HS) // vw
                    hs_per_vh = vw // c.HS
                    col0 = 2 * c.H * c.HS
                    vjobs = [(tcb, vh) for tcb in range(c.NTC)
                             for vh in range(nvh)]
                    for grp in _chunks(vjobs, 4):
                        pss = dict(zip(grp, psum_group(
                            [f"v{j[0]}_{j[1]}" for j in grp])))
                        wts = {}
                        for vh in sorted({vh for _, vh in grp}):
                            wt = wpool.tile([P, c.NEC, vw], BF16,
                                            tag="wblk", name="wt")
                            nc.sync.dma_start(
                                wt[:],
                                wqkv_t[l, :, col0 + vh * vw:
                                       col0 + (vh + 1) * vw]
                                .rearrange("(ec p) w -> p ec w", p=P))
                            wts[vh] = wt
                        for ec in range(c.NEC):
                            for tcb, vh in grp:
                                nc.tensor.matmul(
                                    out=pss[(tcb, vh)][:, :vw],
                                    lhsT=src[:, ec, tcb * P:(tcb + 1) * P],
                                    rhs=wts[vh][:, ec, :],
                                    start=(ec == 0), stop=(ec == c.NEC - 1))
                        for tcb, vh in grp:
                            for hh in range(hs_per_vh):
                                h = vh * hs_per_vh + hh
                                drain_copy(
                                    dst[:, tcb, h * c.HP:h * c.HP + c.HS],
                                    pss[(tcb, vh)][:, hh * c.HS:
                                                   (hh + 1) * c.HS])

                # ============ layers ============
                for l in range(c.L):
                    hT = sb.tile([P, c.NEC, c.TC], BF16, tag="hT", name="hT")
                    layernorm(xT, ln1g_t[l], ln1b_t[l], hT)

                    # ---- pair exchange of hT via zero-slot ReduceScatter ----
                    if "coll" not in ablate:
                        nc.gpsimd.indirect_dma_start(
                            out=agi_d[:],
                            out_offset=bass.IndirectOffsetOnAxis(
                                ap=pidx_sb[:], axis=0),
                            in_=hT[:].rearrange("p a b -> p (a b)"),
                            in_offset=None)
                        nc.gpsimd.collective_compute(
                            "ReduceScatter", OP.add, replica_groups=PAIRS,
                            ins=[agi_d[:].rearrange("a b -> (a b)")],
                            outs=[ago_d[:]])
                    else:
                        nc.sync.dma_start(
                            ago_d[:].rearrange("(p w) -> p w", p=P),
                            hT[:].rearrange("p a b -> p (a b)"))

                    # ---- own-half projections overlap the collective ----
                    kf_own = sb.tile([P, c.NHP, c.TC], BF16, tag="kfo",
                                     name="kf_own")
                    qT = sb.tile([P, c.NHP, c.TC], BF16, tag="qT", name="qT")
                    vf_own = sb.tile([P, c.NTC, c.vrow], BF16, tag="vfo",
                                     name="vf_own")
                    vf_par = sb.tile([P, c.NTC, c.vrow], BF16, tag="vfp",
                                     name="vf_par")
                    for h in range(c.H):
                        for v_ in (vf_own, vf_par):
                            nc.vector.memset(
                                v_[:, :, h * c.HP + c.HS:
                                   h * c.HP + c.HS + 1], 1.0)
                    if "qkv" in ablate:
                        nc.vector.memset(qT[:], 0.0078125)
                        nc.vector.memset(kf_own[:], 0.0078125)
                    else:
                        qk_proj(1, kf_own, hT, l)
                        qk_proj(0, qT, hT, l)
                        v_proj(vf_own, hT, l)

                    # ---- partner h arrives; recompute partner k/v ----
                    # hf load goes through gpsimd (SWDGE): its wait on the
                    # collective must not head-of-line-block the sync DGE
                    # queue, where it would stall the partner weight
                    # prefetches that have no ago dependency.
                    hf = sb.tile([P, c.NEC, c.TC], BF16, tag="hfp",
                                 name="hfp")
                    nc.gpsimd.dma_start(
                        hf[:], ago_d[:].rearrange("(p ec t) -> p ec t",
                                                  p=P, ec=c.NEC))
                    kf_par = sb.tile([P, c.NHP, c.TC], BF16, tag="kfp",
                                     name="kf_par")
                    if "qkv" in ablate:
                        nc.vector.memset(kf_par[:], 0.0078125)
                    else:
                        qk_proj(1, kf_par, hf, l)
                        v_proj(vf_par, hf, l)

                    # ---- attention ----
                    # q chunk j attends own chunks i<=j (diag: tril) and
                    # partner chunks i<=j (diag: msk01 0/1 per core).
                    attT = sb.tile([P, c.NHP, c.TC], BF16, tag="attT",
                                   name="attT")
                    if "attn" in ablate:
                        nc.vector.memset(attT[:], 0.00390625)
                    # Wide-N blocks: per (slot s, key chunk i) one score
                    # matmul over the whole valid q range [i*128, 512) and
                    # one AV matmul accumulating the same range. 16 matmuls
                    # per head instead of 40 -- the N=128 version is
                    # LDWEIGHTS-bound on silicon (~107ns weight load vs
                    # ~53ns stream per matmul). Each block's first 128 cols
                    # are its diagonal (q chunk i): tril for own, 0/1 for
                    # partner (both slices of the mdiag input).
                    TILES = [
                        [(0, 0, 512), (1, 0, 512)],
                        [(0, 1, 384), (1, 1, 384)],
                        [(0, 2, 256), (1, 2, 256),
                         (0, 3, 128), (1, 3, 128)],
                    ]
                    nav = sum(len(tb) for tb in TILES)
                    for h in range(0 if "attn" in ablate else c.H):
                        hp, hb = divmod(h, c.HPP)
                        p0 = hb * c.HS
                        att_ps = psum(name="att_ps")
                        av_i = 0
                        for tb in TILES:
                            offs, o = [], 0
                            for (s, i, n) in tb:
                                offs.append(o)
                                o += n
                            s4 = psum4(name="s4")
                            for (s, i, n), off in zip(tb, offs):
                                kfx = kf_own if s == 0 else kf_par
                                nc.tensor.matmul(
                                    out=s4[:, off:off + n],
                                    lhsT=kfx[p0:p0 + c.HS, hp,
                                             i * P:(i + 1) * P],
                                    rhs=qT[p0:p0 + c.HS, hp,
                                           i * P:i * P + n],
                                    start=True, stop=True)
                            ex = xpool.tile([P, 8 * P], BF16, tag="ex",
                                            name="ex", bufs=3)
                            if "exp" in ablate:
                                nc.vector.tensor_copy(
                                    out=ex[:, :o], in_=s4[:, :o])
                            else:
                                nc.scalar.activation(
                                    ex[:, :o], s4[:, :o],
                                    AF.Exp, scale=c.scale)
                                for (s, i, n), off in zip(tb, offs):
                                    m = mdiag_sb[:, i, s * P:(s + 1) * P]
                                    nc.vector.tensor_tensor(
                                        out=ex[:, off:off + P],
                                        in0=ex[:, off:off + P],
                                        in1=m, op=OP.mult)
                            for (s, i, n), off in zip(tb, offs):
                                vfx = vf_own if s == 0 else vf_par
                                nc.tensor.matmul(
                                    out=att_ps[:c.HP, i * P:i * P + n],
                                    lhsT=vfx[:, i,
                                             h * c.HP:(h + 1) * c.HP],
                                    rhs=ex[:, off:off + n],
                                    start=(av_i == 0),
                                    stop=(av_i == nav - 1),
                                    skip_group_check=True)
                                av_i += 1
                        rec = xpool.tile([1, c.TC], F32, tag="rec",
                                         name="rec", bufs=2)
                        nc.vector.reciprocal(rec[:], att_ps[c.HS:c.HP, :c.TC])
                        recb = xpool.tile([c.HS, c.TC], F32, tag="recb",
                                          name="recb", bufs=2)
                        nc.gpsimd.partition_broadcast(recb[:], rec[:])
                        nc.vector.tensor_tensor(
                            out=attT[p0:p0 + c.HS, hp, :],
                            in0=att_ps[:c.HS, :c.TC],
                            in1=recb[:], op=OP.mult)

                    # ---- Wo projection + bo + residual ----
                    bob = sb.tile([P, c.NEC], F32, tag="bob", name="bob",
                                  bufs=2)
                    nc.sync.dma_start(
                        bob[:], bo_t[l].rearrange("(ec p) -> p ec", p=P))
                    for eos in _chunks(range(c.NEC), 4):
                        pss = dict(zip(eos, psum_group(
                            [f"wo{eo}" for eo in eos])))
                        w = len(eos) * P
                        wt = wpool.tile([P, c.NEC, w], BF16,
                                        tag="wblk", name="wt")
                        nc.sync.dma_start(
                            wt[:], wo_t[l, :, eos[0] * P:eos[0] * P + w]
                            .rearrange("(ec p) w -> p ec w", p=P))
                        for ec in range(c.NEC):
                            for j, eo in enumerate(eos):
                                nc.tensor.matmul(
                                    out=pss[eo][:, :c.TC],
                                    lhsT=wt[:, ec, j * P:(j + 1) * P],
                                    rhs=attT[:, ec, :],
                                    start=(ec == 0), stop=(ec == c.NEC - 1))
                        for eo in eos:
                            nc.vector.scalar_tensor_tensor(
                                out=xT[:, eo, :], in0=pss[eo][:, :c.TC],
                                scalar=bob[:, eo:eo + 1],
                                in1=xT[:, eo, :], op0=OP.add, op1=OP.add)

                    # ---- LN2 + FFN ----
                    h2T = sb.tile([P, c.NEC, c.TC], BF16, tag="hT", name="h2T")
                    layernorm(xT, ln2g_t[l], ln2b_t[l], h2T)

                    b1b = sb.tile([P, c.NFC], F32, tag="b1b", name="b1b",
                                  bufs=2)
                    nc.sync.dma_start(
                        b1b[:], b1_t[l].rearrange("(fc p) -> p fc", p=P))
                    uT = sb.tile([P, c.NFC, c.TC], BF16, tag="uT", name="uT")
                    if "ffn" in ablate:
                        nc.vector.memset(uT[:], 0.0078125)
                    for fcs in ([] if "ffn" in ablate
                                else _chunks(range(c.NFC), 4)):
                        pss = dict(zip(fcs, psum_group(
                            [f"u{fc}" for fc in fcs])))
                        w = len(fcs) * P
                        wt = wpool.tile([P, c.NEC, w], BF16,
                                        tag="wblk", name="wt")
                        nc.sync.dma_start(
                            wt[:], w1_t[l, :, fcs[0] * P:fcs[0] * P + w]
                            .rearrange("(ec p) w -> p ec w", p=P))
                        for ec in range(c.NEC):
                            for j, fc in enumerate(fcs):
                                nc.tensor.matmul(
                                    out=pss[fc][:, :c.TC],
                                    lhsT=wt[:, ec, j * P:(j + 1) * P],
                                    rhs=h2T[:, ec, :],
                                    start=(ec == 0), stop=(ec == c.NEC - 1))
                        for fc in fcs:
                            # relu(x + b1): alternate ACT / DVE
                            if fc % 2 == 0:
                                nc.scalar.activation(
                                    uT[:, fc, :], pss[fc][:, :c.TC],
                                    AF.Relu, bias=b1b[:, fc:fc + 1])
                            else:
                                nc.vector.tensor_scalar(
                                    out=uT[:, fc, :], in0=pss[fc][:, :c.TC],
                                    scalar1=b1b[:, fc:fc + 1], scalar2=0.0,
                                    op0=OP.add, op1=OP.max)

                    b2b = sb.tile([P, c.NEC], F32, tag="bob", name="b2b",
                                  bufs=2)
                    nc.sync.dma_start(
                        b2b[:], b2_t[l].rearrange("(ec p) -> p ec", p=P))
                    for eos in ([] if "ffn" in ablate
                                else _chunks(range(c.NEC), 4)):
                        pss = dict(zip(eos, psum_group(
                            [f"y{eo}" for eo in eos])))
                        w = len(eos) * P
                        for kcs in _chunks(range(c.NFC), 8):
                            wt = wpool.tile([P, len(kcs), w], BF16,
                                            tag="wblk", name="wt")
                            nc.sync.dma_start(
                                wt[:], w2_t[l, kcs[0] * P:
                                            (kcs[-1] + 1) * P,
                                            eos[0] * P:eos[0] * P + w]
                                .rearrange("(kc p) w -> p kc w", p=P))
                            for ki, kc in enumerate(kcs):
                                for j, eo in enumerate(eos):
                                    nc.tensor.matmul(
                                        out=pss[eo][:, :c.TC],
                                        lhsT=wt[:, ki, j * P:(j + 1) * P],
                                        rhs=uT[:, kc, :],
                                        start=(kc == 0),
                                        stop=(kc == c.NFC - 1))
                        for eo in eos:
                            nc.vector.scalar_tensor_tensor(
                                out=xT[:, eo, :], in0=pss[eo][:, :c.TC],
                                scalar=b2b[:, eo:eo + 1],
                                in1=xT[:, eo, :], op0=OP.add, op1=OP.add)

                # ============ final LN + lm_head ============
                xlnT = sb.tile([P, c.NEC, c.TC], BF16, tag="hT", name="xlnT")
                layernorm(xT, lnfg_t, lnfb_t, xlnT)

                vcs = [] if "lmhead" in ablate else c.vchunks
                for v0, wv in vcs:
                    pss = dict(zip(range(c.NTC), psum_group(
                        [f"lg{t}" for t in range(c.NTC)])))
                    wt = wpool.tile([P, c.NEC, 512], BF16, tag="wblk",
                                    name="wt")
                    nc.sync.dma_start(
                        wt[:, :, :wv], wh_t[:, v0:v0 + wv]
                        .rearrange("(ec p) w -> p ec w", p=P))
                    for ec in range(c.NEC):
                        for tcb in range(c.NTC):
                            nc.tensor.matmul(
                                out=pss[tcb][:, :wv],
                                lhsT=xlnT[:, ec, tcb * P:(tcb + 1) * P],
                                rhs=wt[:, ec, :wv],
                                start=(ec == 0), stop=(ec == c.NEC - 1))
                    lg = xpool.tile([P, c.NTC, 512], BF16, tag="lg",
                                    name="lg", bufs=2)
                    for tcb in range(c.NTC):
                        drain_copy(lg[:, tcb, :wv], pss[tcb][:, :wv])
                    nc.sync.dma_start(
                        out_t[:, v0:v0 + wv]
                        .rearrange("(tcb p) w -> p tcb w", p=P),
                        lg[:, :, :wv])

    nc.compile()
    return nc


# ----------------------------------------------------------------------------
# host side
# ----------------------------------------------------------------------------

def prep_inputs(c: Cfg, inputs):
    """Build the 8 per-core input maps from the full model inputs."""
    bf = ml_dtypes.bfloat16
    f32 = np.float32

    idx = np.asarray(inputs["idx"]).astype(np.int32)
    temb = np.asarray(inputs["tok_emb"], f32).astype(bf)
    pos = np.asarray(inputs["pos_emb"], f32)
    Wq, Wk, Wv = (np.asarray(inputs[k], f32) for k in ("Wq", "Wk", "Wv"))
    EHH = c.H * c.HS
    wqkv = np.ascontiguousarray(np.concatenate(
        [w.transpose(0, 2, 1, 3).reshape(c.L, c.E, EHH)
         for w in (Wq, Wk, Wv)], axis=2).astype(bf))

    kk = np.arange(P)[:, None]
    qq = np.arange(P)[None, :]
    tril = (kk <= qq).astype(np.float32)

    shared = {
        "temb": temb, "wqkv": wqkv,
        "wo": np.asarray(inputs["Wo"], f32).astype(bf),
        "w1": np.asarray(inputs["W1"], f32).astype(bf),
        "w2": np.asarray(inputs["W2"], f32).astype(bf),
        "wh": np.asarray(inputs["Wh"], f32).astype(bf),
        "bo": np.asarray(inputs["bo"], f32),
        "ln1g": np.asarray(inputs["ln1_g"], f32),
        "ln1b": np.asarray(inputs["ln1_b"], f32),
        "ln2g": np.asarray(inputs["ln2_g"], f32),
        "ln2b": np.asarray(inputs["ln2_b"], f32),
        "b1": np.asarray(inputs["b1"], f32),
        "b2": np.asarray(inputs["b2"], f32),
        "lnfg": np.asarray(inputs["lnf_g"], f32),
        "lnfb": np.asarray(inputs["lnf_b"], f32),
    }

    in_maps = []
    for core in range(N_CORES):
        b, half = divmod(core, 2)
        own = c.CHUNKS[half]
        par = c.CHUNKS[1 - half]
        tok = np.concatenate([np.arange(g * P, (g + 1) * P) for g in own])
        # per q chunk j: [tril | partner-diag 0/1] masking the two
        # adjacent diagonal blocks of the score layout
        mdiag = np.zeros((c.NTC, P, 2 * P), np.float32)
        for j in range(c.NTC):
            mdiag[j, :, 0:P] = tril
            mdiag[j, :, P:] = 1.0 if par[j] < own[j] else 0.0
        in_maps.append(dict(
            shared,
            idx=np.ascontiguousarray(idx[b, tok]),
            posT=np.ascontiguousarray(pos[tok].T),
            mdiag=np.ascontiguousarray(mdiag.astype(bf)),
            pidx=((1 - half) * P + np.arange(P)).astype(np.int32),
            oidx=(half * P + np.arange(P)).astype(np.int32),
        ))
    return in_maps


_CACHE = {}


def _get_program():
    if "nc" not in _CACHE:
        _CACHE["cfg"] = Cfg()
        _CACHE["nc"] = build_program(_CACHE["cfg"])
    return _CACHE["nc"], _CACHE["cfg"]


def kernel(**inputs) -> np.ndarray:
    nc, c = _get_program()
    in_maps = prep_inputs(c, inputs)
    res = bass_utils.run_bass_kernel_spmd(
        nc, in_maps, core_ids=list(range(N_CORES)))
    out = np.empty((c.B, c.T, c.V), np.float32)
    for core in range(N_CORES):
        b, half = divmod(core, 2)
        o = res.results[core]["out"].astype(np.float32)
        for i, g in enumerate(c.CHUNKS[half]):
            out[b, g * P:(g + 1) * P] = o[i * P:(i + 1) * P]
    out += np.asarray(inputs["bh"], np.float32)
    return out


# revision 37
# speedup vs baseline: 1.7412x; 1.3303x over previous
"""Trainium2 Bass kernel for a 4-layer decoder transformer (B4 T1024 E1024 H16
hs64 F4096 V32000) on 8 NeuronCores.

Sharding: batch(4) x sequence-half(2). Core c handles batch b=c//2 and the
causal-interleaved token chunks CHUNKS[c%2]. The residual stream lives in
SBUF transposed (xT: [E, 512], E on partitions) so every matmul has its
contraction dim on partitions.

Cross-core exchange (v2): instead of AllGathering k|v (2MB in, 4MB out per
core), each core sends only its LN1 output hT (1MB) to its pair partner and
recomputes the partner's k/v locally. The exchange is a pair
ReduceScatter(add): each core scatters hT into the PARTNER's slot of agi
(own slot pre-zeroed once), so RS yields exactly the partner's hT -- half
the collective bytes of an AllGather of the same payload, and the
per-core-different slot offset is handled by an indirect DMA driven by a
host-supplied row-index input (keeps the program SPMD-uniform).

Attention indexes keys as (own/partner, chunk) triangular blocks: q chunk j
attends own chunks i<=j (diagonal tril-masked) and partner chunks i<=j
(diagonal 0/1-masked per core, input msk01). This is uniform across cores
and does the same 20 score blocks per head as the previous POSF scheme.

PSUM layout: tag "mm" = 4 rotating single-bank tiles, tag "mm4" = [128,
1024] 2-bank tiles (bufs=2) used for score batches so exp runs as 3 big ACT
ops per head. All matmul phases use 4-bank accumulation blocks so
consecutive blocks double-buffer. Elementwise psum drains alternate between
DVE and ACT to balance engine load. LN rstd = exp(-0.5*ln(var+eps)) keeps
every ACT call in the natural_log_exp table set (no table reloads).
"""

import numpy as np
import ml_dtypes

import concourse.bass as bass
import concourse.bacc as bacc
import concourse.mybir as mybir
import concourse.tile as tile
from concourse import bass_utils
from concourse.masks import make_identity

F32 = mybir.dt.float32
F32R = mybir.dt.float32r
BF16 = mybir.dt.bfloat16
I32 = mybir.dt.int32
AF = mybir.ActivationFunctionType
OP = mybir.AluOpType
P = 128

N_CORES = 8
PAIRS = [[0, 1], [2, 3], [4, 5], [6, 7]]


def _chunks(seq, n):
    seq = list(seq)
    return [seq[i:i + n] for i in range(0, len(seq), n)]


class Cfg:
    def __init__(self, B=4, T=1024, E=1024, H=16, HS=64, L=4, F=4096, V=32000):
        self.B, self.T, self.E, self.H, self.HS = B, T, E, H, HS
        self.L, self.F, self.V = L, F, V
        self.TC = T // 2                    # tokens per core
        self.NEC = E // P                   # E chunks (partition tiles)
        self.NTC = self.TC // P             # local token chunks
        self.NFC = F // P                   # FFN hidden chunks
        self.HPP = P // HS                  # heads per 128-partition tile
        self.NHP = (H * HS) // P            # head-pair tiles
        self.HP = HS + 1                    # augmented per-head stride in v
        self.scale = 1.0 / (E ** 0.5)
        self.hrow = self.NEC * self.TC      # per-partition hT row (4096)
        self.vchunks = []
        v0 = 0
        while v0 < V:
            self.vchunks.append((v0, min(512, V - v0)))
            v0 += 512
        self.vrow = H * self.HP
        # Causal-interleaved token chunks: core half h owns global 128-token
        # chunks CHUNKS[h] (in this local order). Both lists ascend, so
        # own-vs-own causality is exactly i<=j with tril on the diagonal,
        # and partner chunk i is valid for q chunk j iff
        # CHUNKS[partner][i] < CHUNKS[own][j] -- which holds for all i<j and
        # alternates on the diagonal (host input msk01).
        self.CHUNKS = [[0, 3, 4, 7], [1, 2, 5, 6]]


def build_program(c: Cfg, reps: int = 1, ablate=()):
    # Every ACT call here uses only {Exp, Ln, Relu, Copy, Identity}, all of
    # which live in the natural_log_exp_and_others table set. The stock
    # table chooser maps Exp->exp_and_others and Ln->natural_log, inserting
    # a ~1.3us table load per switch (2 per layernorm). Restrict the
    # eligible sets (ids preserved) during this build so one load serves
    # the whole program.
    import concourse.bacc as _bacc_mod
    _orig_tables = _bacc_mod.get_activation_tables

    def _only_ln_exp(arch):
        tabs = _orig_tables(arch)
        return {name: (s if name == "natural_log_exp_and_others" else set())
                for name, s in tabs.items()}

    _bacc_mod.get_activation_tables = _only_ln_exp
    try:
        return _build_program_inner(c, reps, ablate)
    finally:
        _bacc_mod.get_activation_tables = _orig_tables


def _build_program_inner(c: Cfg, reps: int = 1, ablate=()):
    nc = bacc.Bacc("TRN2", target_bir_lowering=False, debug=False,
                   num_devices=N_CORES)

    # ---- DRAM I/O ----
    dt_ = nc.dram_tensor
    idx_t = dt_("idx", [c.TC], I32, kind="ExternalInput").ap()
    temb_t = dt_("temb", [c.V, c.E], BF16, kind="ExternalInput").ap()
    posT_t = dt_("posT", [c.E, c.TC], F32, kind="ExternalInput").ap()
    wqkv_t = dt_("wqkv", [c.L, c.E, 3 * c.H * c.HS], BF16,
                 kind="ExternalInput").ap()
    wo_t = dt_("wo", [c.L, c.E, c.E], BF16, kind="ExternalInput").ap()
    bo_t = dt_("bo", [c.L, c.E], F32, kind="ExternalInput").ap()
    ln1g_t = dt_("ln1g", [c.L, c.E], F32, kind="ExternalInput").ap()
    ln1b_t = dt_("ln1b", [c.L, c.E], F32, kind="ExternalInput").ap()
    ln2g_t = dt_("ln2g", [c.L, c.E], F32, kind="ExternalInput").ap()
    ln2b_t = dt_("ln2b", [c.L, c.E], F32, kind="ExternalInput").ap()
    w1_t = dt_("w1", [c.L, c.E, c.F], BF16, kind="ExternalInput").ap()
    b1_t = dt_("b1", [c.L, c.F], F32, kind="ExternalInput").ap()
    w2_t = dt_("w2", [c.L, c.F, c.E], BF16, kind="ExternalInput").ap()
    b2_t = dt_("b2", [c.L, c.E], F32, kind="ExternalInput").ap()
    lnfg_t = dt_("lnfg", [c.E], F32, kind="ExternalInput").ap()
    lnfb_t = dt_("lnfb", [c.E], F32, kind="ExternalInput").ap()
    wh_t = dt_("wh", [c.E, c.V], BF16, kind="ExternalInput").ap()
    mdiag_t = dt_("mdiag", [c.NTC, P, 2 * P], BF16,
                  kind="ExternalInput").ap()
    pidx_t = dt_("pidx", [P], I32, kind="ExternalInput").ap()
    oidx_t = dt_("oidx", [P], I32, kind="ExternalInput").ap()
    # logits written bf16; host upcasts to f32 and adds bh there
    out_t = dt_("out", [c.TC, c.V], BF16, kind="ExternalOutput").ap()

    with tile.TileContext(nc) as tc:
        with tc.tile_pool(name="sb", bufs=1) as sb, \
             tc.tile_pool(name="wpool", bufs=4) as wpool, \
             tc.tile_pool(name="xpool", bufs=2) as xpool, \
             tc.tile_pool(name="ps", bufs=4, space="PSUM") as ps, \
             tc.tile_pool(name="dram", bufs=1, space="DRAM") as dram:

            def psum(shape=None, dtype=F32, name="mm"):
                return ps.tile(shape or [P, 512], dtype, tag="mm", name=name)

            def psum4(name="mm4"):
                return ps.tile([P, 2 * 512], F32, tag="mm4", name=name,
                               bufs=2)

            # Projection phases need 4 [P,512] accumulators per output
            # group; alternating groups between the mm banks and the mm4
            # banks (idle outside attention) lets group g+1's matmuls run
            # while group g's psum->sbuf copies drain.
            pgc = [0]

            def psum_group(names):
                pgc[0] += 1
                if pgc[0] % 2 == 0:
                    return [psum(name=nm) for nm in names]
                a = psum4(name="pgA")
                b = psum4(name="pgB")
                views = [a[:, 0:512], a[:, 512:1024],
                         b[:, 0:512], b[:, 512:1024]]
                return views[:len(names)]

            # psum->sbuf drains alternate DVE/ACT to balance engine load
            cpc = [0]

            def drain_copy(out, in_):
                cpc[0] += 1
                if cpc[0] % 2 == 0:
                    nc.scalar.copy(out, in_)
                else:
                    nc.vector.tensor_copy(out=out, in_=in_)

            # ---- constants ----
            ones_bf = sb.tile([P, 1], BF16, tag="ones_bf", name="ones_bf")
            nc.vector.memset(ones_bf[:], 1.0)
            eps_sb = sb.tile([1, 1], F32, tag="eps", name="eps_sb")
            nc.vector.memset(eps_sb[:], 1e-5)
            ident = sb.tile([P, P], BF16, tag="ident", name="ident")
            make_identity(nc, ident[:])
            mdiag_sb = sb.tile([P, c.NTC, 2 * P], BF16, tag="mdiag",
                               name="mdiag_sb")
            for j in range(c.NTC):
                nc.sync.dma_start(mdiag_sb[:, j, :], mdiag_t[j])
            pidx_sb = sb.tile([P, 1], I32, tag="pidx", name="pidx_sb")
            nc.sync.dma_start(pidx_sb[:], pidx_t.rearrange("(p o) -> p o", o=1))
            oidx_sb = sb.tile([P, 1], I32, tag="oidx", name="oidx_sb")
            nc.sync.dma_start(oidx_sb[:], oidx_t.rearrange("(p o) -> p o", o=1))
            idx_sb = sb.tile([P, c.NTC], I32, tag="idx", name="idx_sb")
            nc.sync.dma_start(idx_sb[:], idx_t.rearrange("(tc p) -> p tc", p=P))

            # ---- collective staging: agi [2P, hrow]; own slot rows are
            # zeroed once so the pair ReduceScatter(add) yields exactly the
            # partner's hT in ago.
            agi_d = dram.tile([2 * P, c.hrow], BF16, tag="agi", name="agi")
            ago_d = dram.tile([P * c.hrow], BF16, tag="ago", name="ago")
            zT = sb.tile([P, c.hrow], BF16, tag="zT", name="zT")
            nc.vector.memset(zT[:], 0.0)
            nc.gpsimd.indirect_dma_start(
                out=agi_d[:], out_offset=bass.IndirectOffsetOnAxis(
                    ap=oidx_sb[:], axis=0),
                in_=zT[:], in_offset=None)

            for _rep in range(reps):
                # ---- residual stream xT[e, t] (f32), seeded with pos^T ----
                xT = sb.tile([P, c.NEC, c.TC], F32, tag="xT", name="xT")
                nc.sync.dma_start(
                    xT[:], posT_t.rearrange("(ec p) t -> p ec t", p=P))

                # ---- embedding gather + transpose ----
                for tcb in range(c.NTC):
                    emb = xpool.tile([P, c.E], BF16, tag="emb", name="emb")
                    if "gather" in ablate:
                        nc.sync.dma_start(emb[:],
                                          temb_t[tcb * P:(tcb + 1) * P, :])
                    else:
                        nc.gpsimd.indirect_dma_start(
                            out=emb[:], out_offset=None, in_=temb_t,
                            in_offset=bass.IndirectOffsetOnAxis(
                                ap=idx_sb[:, tcb:tcb + 1], axis=0))
                    for ec in range(c.NEC):
                        tps = psum([P, P], BF16, name="tps")
                        nc.tensor.transpose(
                            out=tps[:], in_=emb[:, ec * P:(ec + 1) * P],
                            identity=ident[:])
                        tpf = xpool.tile([P, P], F32, tag="tpf", name="tpf")
                        nc.vector.tensor_copy(out=tpf[:], in_=tps[:])
                        sl = xT[:, ec, tcb * P:(tcb + 1) * P]
                        nc.vector.tensor_tensor(out=sl, in0=sl, in1=tpf[:],
                                                op=OP.add)

                # ---- layernorm: xT -> out_bf (bf16 [P, NEC, TC]) ----
                # rstd = exp(-0.5*ln(var+eps)) stays in the exp/ln ACT set.
                def layernorm(xT, g_dram, b_dram, out_bf):
                    gb = sb.tile([P, 2 * c.NEC], F32, tag="gains", name="gb",
                                 bufs=2)
                    nc.sync.dma_start(
                        gb[:, 0:c.NEC], g_dram.rearrange("(ec p) -> p ec", p=P))
                    nc.sync.dma_start(
                        gb[:, c.NEC:], b_dram.rearrange("(ec p) -> p ec", p=P))
                    sum_ps = psum([1, c.TC], name="ln_sum")
                    sq_ps = psum([1, c.TC], name="ln_sq")
                    for ec in range(c.NEC):
                        xbf = xpool.tile([P, c.TC], BF16, tag="xbf", name="xbf")
                        nc.vector.tensor_copy(out=xbf[:], in_=xT[:, ec, :])
                        nc.tensor.matmul(out=sum_ps[:], lhsT=ones_bf[:],
                                         rhs=xbf[:], start=(ec == 0),
                                         stop=(ec == c.NEC - 1))
                        xsq = xpool.tile([P, c.TC], BF16, tag="xsq", name="xsq")
                        if ec % 2 == 0:
                            nc.scalar.activation(xsq[:], xbf[:], AF.Square)
                        else:
                            nc.vector.tensor_tensor(out=xsq[:], in0=xbf[:],
                                                    in1=xbf[:], op=OP.mult)
                        nc.tensor.matmul(out=sq_ps[:], lhsT=ones_bf[:],
                                         rhs=xsq[:], start=(ec == 0),
                                         stop=(ec == c.NEC - 1))
                    stats = xpool.tile([1, 3 * c.TC], F32, tag="stats",
                                       name="stats", bufs=1)
                    mean = stats[:, 0:c.TC]
                    var = stats[:, c.TC:2 * c.TC]
                    rstd = stats[:, 2 * c.TC:]
                    inv_e = 1.0 / c.E
                    # var+eps = (sq - sum^2/E)/E + eps with /E and eps folded
                    # into Ln's scale/bias; rstd = exp(-0.5*ln(var+eps)).
                    m2 = xpool.tile([1, c.TC], F32, tag="m2", name="m2")
                    nc.vector.tensor_scalar_mul(mean, sum_ps[:], inv_e)
                    nc.vector.tensor_tensor(out=m2[:], in0=mean, in1=mean,
                                            op=OP.mult)
                    nc.vector.tensor_scalar_mul(var, sq_ps[:], inv_e)
                    nc.vector.tensor_tensor(out=var, in0=var, in1=m2[:],
                                            op=OP.subtract)
                    nc.scalar.activation(var, var, AF.Ln, bias=eps_sb[:])
                    nc.scalar.activation(rstd, var, AF.Exp, scale=-0.5)
                    mrb = xpool.tile([P, 2, c.TC], F32, tag="mrb",
                                     name="mrb", bufs=1)
                    nc.gpsimd.partition_broadcast(mrb[:, 0, :], mean)
                    nc.gpsimd.partition_broadcast(mrb[:, 1, :], rstd)
                    for ec in range(c.NEC):
                        tmp = xpool.tile([P, c.TC], F32, tag="lntmp",
                                         name="lntmp")
                        nc.vector.tensor_tensor(out=tmp[:], in0=xT[:, ec, :],
                                                in1=mrb[:, 0, :],
                                                op=OP.subtract)
                        nc.vector.tensor_tensor(out=tmp[:], in0=tmp[:],
                                                in1=mrb[:, 1, :], op=OP.mult)
                        nc.vector.tensor_scalar(
                            out=out_bf[:, ec, :], in0=tmp[:],
                            scalar1=gb[:, ec:ec + 1],
                            scalar2=gb[:, c.NEC + ec:c.NEC + ec + 1],
                            op0=OP.mult, op1=OP.add)

                # qk-style projection: dst[feat_chunk, tok] from src hT-like
                def qk_proj(which, dst, src, l):
                    col0 = which * c.H * c.HS
                    for fcs in _chunks(range(c.NHP), 4):
                        pss = dict(zip(fcs, psum_group(
                            [f"qk{fc}" for fc in fcs])))
                        w = len(fcs) * P
                        wt = wpool.tile([P, c.NEC, w], BF16,
                                        tag="wblk", name="wt")
                        nc.sync.dma_start(
                            wt[:],
                            wqkv_t[l, :, col0 + fcs[0] * P:
                                   col0 + fcs[0] * P + w]
                            .rearrange("(ec p) w -> p ec w", p=P))
                        for ec in range(c.NEC):
                            for j, fc in enumerate(fcs):
                                nc.tensor.matmul(
                                    out=pss[fc][:, :c.TC],
                                    lhsT=wt[:, ec, j * P:(j + 1) * P],
                                    rhs=src[:, ec, :],
                                    start=(ec == 0),
                                    stop=(ec == c.NEC - 1))
                        for fc in fcs:
                            drain_copy(dst[:, fc, :], pss[fc][:, :c.TC])

                # v projection (natural [tok, vrow] layout, ones col per head)
                def v_proj(dst, src, l):
                    vw = min(512, c.H * c.HS)
                    nvh = (c.H * c.HS) // vw
                    hs_per_vh = vw // c.HS
                    col0 = 2 * c.H * c.HS
                    vjobs = [(tcb, vh) for tcb in range(c.NTC)
                             for vh in range(nvh)]
                    for grp in _chunks(vjobs, 4):
                        pss = dict(zip(grp, psum_group(
                            [f"v{j[0]}_{j[1]}" for j in grp])))
                        wts = {}
                        for vh in sorted({vh for _, vh in grp}):
                            wt = wpool.tile([P, c.NEC, vw], BF16,
                                            tag="wblk", name="wt")
                            nc.sync.dma_start(
                                wt[:],
                                wqkv_t[l, :, col0 + vh * vw:
                                       col0 + (vh + 1) * vw]
                                .rearrange("(ec p) w -> p ec w", p=P))
                            wts[vh] = wt
                        for ec in range(c.NEC):
                            for tcb, vh in grp:
                                nc.tensor.matmul(
                                    out=pss[(tcb, vh)][:, :vw],
                                    lhsT=src[:, ec, tcb * P:(tcb + 1) * P],
                                    rhs=wts[vh][:, ec, :],
                                    start=(ec == 0), stop=(ec == c.NEC - 1))
                        for tcb, vh in grp:
                            for hh in range(hs_per_vh):
                                h = vh * hs_per_vh + hh
                                drain_copy(
                                    dst[:, tcb, h * c.HP:h * c.HP + c.HS],
                                    pss[(tcb, vh)][:, hh * c.HS:
                                                   (hh + 1) * c.HS])

                # ============ layers ============
                for l in range(c.L):
                    hT = sb.tile([P, c.NEC, c.TC], BF16, tag="hT", name="hT")
                    layernorm(xT, ln1g_t[l], ln1b_t[l], hT)

                    # ---- pair exchange of hT via zero-slot ReduceScatter ----
                    if "coll" not in ablate:
                        nc.gpsimd.indirect_dma_start(
                            out=agi_d[:],
                            out_offset=bass.IndirectOffsetOnAxis(
                                ap=pidx_sb[:], axis=0),
                            in_=hT[:].rearrange("p a b -> p (a b)"),
                            in_offset=None)
                        nc.gpsimd.collective_compute(
                            "ReduceScatter", OP.add, replica_groups=PAIRS,
                            ins=[agi_d[:].rearrange("a b -> (a b)")],
                            outs=[ago_d[:]])
                    else:
                        nc.sync.dma_start(
                            ago_d[:].rearrange("(p w) -> p w", p=P),
                            hT[:].rearrange("p a b -> p (a b)"))

                    # ---- own-half projections overlap the collective ----
                    kf_own = sb.tile([P, c.NHP, c.TC], BF16, tag="kfo",
                                     name="kf_own")
                    qT = sb.tile([P, c.NHP, c.TC], BF16, tag="qT", name="qT")
                    vf_own = sb.tile([P, c.NTC, c.vrow], BF16, tag="vfo",
                                     name="vf_own")
                    vf_par = sb.tile([P, c.NTC, c.vrow], BF16, tag="vfp",
                                     name="vf_par")
                    for h in range(c.H):
                        for v_ in (vf_own, vf_par):
                            nc.vector.memset(
                                v_[:, :, h * c.HP + c.HS:
                                   h * c.HP + c.HS + 1], 1.0)
                    if "qkv" in ablate:
                        nc.vector.memset(qT[:], 0.0078125)
                        nc.vector.memset(kf_own[:], 0.0078125)
                    else:
                        qk_proj(1, kf_own, hT, l)
                        qk_proj(0, qT, hT, l)
                        v_proj(vf_own, hT, l)

                    # ---- partner h arrives; recompute partner k/v ----
                    # hf load goes through gpsimd (SWDGE): its wait on the
                    # collective must not head-of-line-block the sync DGE
                    # queue, where it would stall the partner weight
                    # prefetches that have no ago dependency.
                    hf = sb.tile([P, c.NEC, c.TC], BF16, tag="hfp",
                                 name="hfp")
                    nc.gpsimd.dma_start(
                        hf[:], ago_d[:].rearrange("(p ec t) -> p ec t",
                                                  p=P, ec=c.NEC))
                    kf_par = sb.tile([P, c.NHP, c.TC], BF16, tag="kfp",
                                     name="kf_par")
                    if "qkv" in ablate:
                        nc.vector.memset(kf_par[:], 0.0078125)
                    else:
                        qk_proj(1, kf_par, hf, l)
                        v_proj(vf_par, hf, l)

                    # ---- attention ----
                    # q chunk j attends own chunks i<=j (diag: tril) and
                    # partner chunks i<=j (diag: msk01 0/1 per core).
                    attT = sb.tile([P, c.NHP, c.TC], BF16, tag="attT",
                                   name="attT")
                    if "attn" in ablate:
                        nc.vector.memset(attT[:], 0.00390625)
                    # Wide-N blocks: per (slot s, key chunk i) one score
                    # matmul over the whole valid q range [i*128, 512) and
                    # one AV matmul accumulating the same range. 16 matmuls
                    # per head instead of 40 -- the N=128 version is
                    # LDWEIGHTS-bound on silicon (~107ns weight load vs
                    # ~53ns stream per matmul). Each block's first 128 cols
                    # are its diagonal (q chunk i): tril for own, 0/1 for
                    # partner (both slices of the mdiag input).
                    TILES = [
                        [(0, 0, 512), (1, 0, 512)],
                        [(0, 1, 384), (1, 1, 384)],
                        [(0, 2, 256), (1, 2, 256),
                         (0, 3, 128), (1, 3, 128)],
                    ]
                    for h in range(0 if "attn" in ablate else c.H):
                        hp, hb = divmod(h, c.HPP)
                        p0 = hb * c.HS
                        att_ps = psum(name="att_ps")
                        exs = {}
                        for tb in TILES:
                            offs, o = [], 0
                            for (s, i, n) in tb:
                                offs.append(o)
                                o += n
                            s4 = psum4(name="s4")
                            for (s, i, n), off in zip(tb, offs):
                                kfx = kf_own if s == 0 else kf_par
                                nc.tensor.matmul(
                                    out=s4[:, off:off + n],
                                    lhsT=kfx[p0:p0 + c.HS, hp,
                                             i * P:(i + 1) * P],
                                    rhs=qT[p0:p0 + c.HS, hp,
                                           i * P:i * P + n],
                                    start=True, stop=True)
                            ex = xpool.tile([P, 8 * P], BF16, tag="ex",
                                            name="ex", bufs=3)
                            nc.scalar.activation(
                                ex[:, :o], s4[:, :o], AF.Exp, scale=c.scale)
                            for (s, i, n), off in zip(tb, offs):
                                m = mdiag_sb[:, i, s * P:(s + 1) * P]
                                nc.vector.tensor_tensor(
                                    out=ex[:, off:off + P],
                                    in0=ex[:, off:off + P],
                                    in1=m, op=OP.mult)
                                exs[(s, i)] = (ex, off)
                        # AV per q chunk (contiguous regular accumulation
                        # groups), reading slices of the three live ex tiles
                        for j in range(c.NTC):
                            nblk = 2 * (j + 1)
                            bi = 0
                            for i in range(j + 1):
                                for s, vfx in ((0, vf_own), (1, vf_par)):
                                    ex, off = exs[(s, i)]
                                    col = off + (j - i) * P
                                    nc.tensor.matmul(
                                        out=att_ps[:c.HP,
                                                   j * P:(j + 1) * P],
                                        lhsT=vfx[:, i,
                                                 h * c.HP:(h + 1) * c.HP],
                                        rhs=ex[:, col:col + P],
                                        start=(bi == 0),
                                        stop=(bi == nblk - 1))
                                    bi += 1
                        rec = xpool.tile([1, c.TC], F32, tag="rec",
                                         name="rec", bufs=2)
                        nc.vector.reciprocal(rec[:], att_ps[c.HS:c.HP, :c.TC])
                        recb = xpool.tile([c.HS, c.TC], F32, tag="recb",
                                          name="recb", bufs=2)
                        nc.gpsimd.partition_broadcast(recb[:], rec[:])
                        nc.vector.tensor_tensor(
                            out=attT[p0:p0 + c.HS, hp, :],
                            in0=att_ps[:c.HS, :c.TC],
                            in1=recb[:], op=OP.mult)

                    # ---- Wo projection + bo + residual ----
                    bob = sb.tile([P, c.NEC], F32, tag="bob", name="bob",
                                  bufs=2)
                    nc.sync.dma_start(
                        bob[:], bo_t[l].rearrange("(ec p) -> p ec", p=P))
                    for eos in _chunks(range(c.NEC), 4):
                        pss = dict(zip(eos, psum_group(
                            [f"wo{eo}" for eo in eos])))
                        w = len(eos) * P
                        wt = wpool.tile([P, c.NEC, w], BF16,
                                        tag="wblk", name="wt")
                        nc.sync.dma_start(
                            wt[:], wo_t[l, :, eos[0] * P:eos[0] * P + w]
                            .rearrange("(ec p) w -> p ec w", p=P))
                        for ec in range(c.NEC):
                            for j, eo in enumerate(eos):
                                nc.tensor.matmul(
                                    out=pss[eo][:, :c.TC],
                                    lhsT=wt[:, ec, j * P:(j + 1) * P],
                                    rhs=attT[:, ec, :],
                                    start=(ec == 0), stop=(ec == c.NEC - 1))
                        for eo in eos:
                            nc.vector.scalar_tensor_tensor(
                                out=xT[:, eo, :], in0=pss[eo][:, :c.TC],
                                scalar=bob[:, eo:eo + 1],
                                in1=xT[:, eo, :], op0=OP.add, op1=OP.add)

                    # ---- LN2 + FFN ----
                    h2T = sb.tile([P, c.NEC, c.TC], BF16, tag="hT", name="h2T")
                    layernorm(xT, ln2g_t[l], ln2b_t[l], h2T)

                    b1b = sb.tile([P, c.NFC], F32, tag="b1b", name="b1b",
                                  bufs=2)
                    nc.sync.dma_start(
                        b1b[:], b1_t[l].rearrange("(fc p) -> p fc", p=P))
                    uT = sb.tile([P, c.NFC, c.TC], BF16, tag="uT", name="uT")
                    if "ffn" in ablate:
                        nc.vector.memset(uT[:], 0.0078125)
                    for fcs in ([] if "ffn" in ablate
                                else _chunks(range(c.NFC), 4)):
                        pss = dict(zip(fcs, psum_group(
                            [f"u{fc}" for fc in fcs])))
                        w = len(fcs) * P
                        wt = wpool.tile([P, c.NEC, w], BF16,
                                        tag="wblk", name="wt")
                        nc.sync.dma_start(
                            wt[:], w1_t[l, :, fcs[0] * P:fcs[0] * P + w]
                            .rearrange("(ec p) w -> p ec w", p=P))
                        for ec in range(c.NEC):
                            for j, fc in enumerate(fcs):
                                nc.tensor.matmul(
                                    out=pss[fc][:, :c.TC],
                                    lhsT=wt[:, ec, j * P:(j + 1) * P],
                                    rhs=h2T[:, ec, :],
                                    start=(ec == 0), stop=(ec == c.NEC - 1))
                        for fc in fcs:
                            # relu(x + b1): alternate ACT / DVE
                            if fc % 2 == 0:
                                nc.scalar.activation(
                                    uT[:, fc, :], pss[fc][:, :c.TC],
                                    AF.Relu, bias=b1b[:, fc:fc + 1])
                            else:
                                nc.vector.tensor_scalar(
                                    out=uT[:, fc, :], in0=pss[fc][:, :c.TC],
                                    scalar1=b1b[:, fc:fc + 1], scalar2=0.0,
                                    op0=OP.add, op1=OP.max)

                    b2b = sb.tile([P, c.NEC], F32, tag="bob", name="b2b",
                                  bufs=2)
                    nc.sync.dma_start(
                        b2b[:], b2_t[l].rearrange("(ec p) -> p ec", p=P))
                    for eos in ([] if "ffn" in ablate
                                else _chunks(range(c.NEC), 4)):
                        pss = dict(zip(eos, psum_group(
                            [f"y{eo}" for eo in eos])))
                        w = len(eos) * P
                        for kcs in _chunks(range(c.NFC), 8):
                            wt = wpool.tile([P, len(kcs), w], BF16,
                                            tag="wblk", name="wt")
                            nc.sync.dma_start(
                                wt[:], w2_t[l, kcs[0] * P:
                                            (kcs[-1] + 1) * P,
                                            eos[0] * P:eos[0] * P + w]
                                .rearrange("(kc p) w -> p kc w", p=P))
                            for ki, kc in enumerate(kcs):
                                for j, eo in enumerate(eos):
                                    nc.tensor.matmul(
                                        out=pss[eo][:, :c.TC],
                                        lhsT=wt[:, ki, j * P:(j + 1) * P],
                                        rhs=uT[:, kc, :],
                                        start=(kc == 0),
                                        stop=(kc == c.NFC - 1))
                        for eo in eos:
                            nc.vector.scalar_tensor_tensor(
                                out=xT[:, eo, :], in0=pss[eo][:, :c.TC],
                                scalar=b2b[:, eo:eo + 1],
                                in1=xT[:, eo, :], op0=OP.add, op1=OP.add)

                # ============ final LN + lm_head ============
                xlnT = sb.tile([P, c.NEC, c.TC], BF16, tag="hT", name="xlnT")
                layernorm(xT, lnfg_t, lnfb_t, xlnT)

                vcs = [] if "lmhead" in ablate else c.vchunks
                for v0, wv in vcs:
                    pss = dict(zip(range(c.NTC), psum_group(
                        [f"lg{t}" for t in range(c.NTC)])))
                    wt = wpool.tile([P, c.NEC, 512], BF16, tag="wblk",
                                    name="wt")
                    nc.sync.dma_start(
                        wt[:, :, :wv], wh_t[:, v0:v0 + wv]
                        .rearrange("(ec p) w -> p ec w", p=P))
                    for ec in range(c.NEC):
                        for tcb in range(c.NTC):
                            nc.tensor.matmul(
                                out=pss[tcb][:, :wv],
                                lhsT=xlnT[:, ec, tcb * P:(tcb + 1) * P],
                                rhs=wt[:, ec, :wv],
                                start=(ec == 0), stop=(ec == c.NEC - 1))
                    lg = xpool.tile([P, c.NTC, 512], BF16, tag="lg",
                                    name="lg", bufs=2)
                    for tcb in range(c.NTC):
                        drain_copy(lg[:, tcb, :wv], pss[tcb][:, :wv])
                    nc.sync.dma_start(
                        out_t[:, v0:v0 + wv]
                        .rearrange("(tcb p) w -> p tcb w", p=P),
                        lg[:, :, :wv])

    nc.compile()
    return nc


# ----------------------------------------------------------------------------
# host side
# ----------------------------------------------------------------------------

def prep_inputs(c: Cfg, inputs):
    """Build the 8 per-core input maps from the full model inputs."""
    bf = ml_dtypes.bfloat16
    f32 = np.float32

    idx = np.asarray(inputs["idx"]).astype(np.int32)
    temb = np.asarray(inputs["tok_emb"], f32).astype(bf)
    pos = np.asarray(inputs["pos_emb"], f32)
    Wq, Wk, Wv = (np.asarray(inputs[k], f32) for k in ("Wq", "Wk", "Wv"))
    EHH = c.H * c.HS
    wqkv = np.ascontiguousarray(np.concatenate(
        [w.transpose(0, 2, 1, 3).reshape(c.L, c.E, EHH)
         for w in (Wq, Wk, Wv)], axis=2).astype(bf))

    kk = np.arange(P)[:, None]
    qq = np.arange(P)[None, :]
    tril = (kk <= qq).astype(np.float32)

    shared = {
        "temb": temb, "wqkv": wqkv,
        "wo": np.asarray(inputs["Wo"], f32).astype(bf),
        "w1": np.asarray(inputs["W1"], f32).astype(bf),
        "w2": np.asarray(inputs["W2"], f32).astype(bf),
        "wh": np.asarray(inputs["Wh"], f32).astype(bf),
        "bo": np.asarray(inputs["bo"], f32),
        "ln1g": np.asarray(inputs["ln1_g"], f32),
        "ln1b": np.asarray(inputs["ln1_b"], f32),
        "ln2g": np.asarray(inputs["ln2_g"], f32),
        "ln2b": np.asarray(inputs["ln2_b"], f32),
        "b1": np.asarray(inputs["b1"], f32),
        "b2": np.asarray(inputs["b2"], f32),
        "lnfg": np.asarray(inputs["lnf_g"], f32),
        "lnfb": np.asarray(inputs["lnf_b"], f32),
    }

    in_maps = []
    for core in range(N_CORES):
        b, half = divmod(core, 2)
        own = c.CHUNKS[half]
        par = c.CHUNKS[1 - half]
        tok = np.concatenate([np.arange(g * P, (g + 1) * P) for g in own])
        # per q chunk j: [tril | partner-diag 0/1] masking the two
        # adjacent diagonal blocks of the score layout
        mdiag = np.zeros((c.NTC, P, 2 * P), np.float32)
        for j in range(c.NTC):
            mdiag[j, :, 0:P] = tril
            mdiag[j, :, P:] = 1.0 if par[j] < own[j] else 0.0
        in_maps.append(dict(
            shared,
            idx=np.ascontiguousarray(idx[b, tok]),
            posT=np.ascontiguousarray(pos[tok].T),
            mdiag=np.ascontiguousarray(mdiag.astype(bf)),
            pidx=((1 - half) * P + np.arange(P)).astype(np.int32),
            oidx=(half * P + np.arange(P)).astype(np.int32),
        ))
    return in_maps


_CACHE = {}


def _get_program():
    if "nc" not in _CACHE:
        _CACHE["cfg"] = Cfg()
        _CACHE["nc"] = build_program(_CACHE["cfg"])
    return _CACHE["nc"], _CACHE["cfg"]


def kernel(**inputs) -> np.ndarray:
    nc, c = _get_program()
    in_maps = prep_inputs(c, inputs)
    res = bass_utils.run_bass_kernel_spmd(
        nc, in_maps, core_ids=list(range(N_CORES)))
    out = np.empty((c.B, c.T, c.V), np.float32)
    for core in range(N_CORES):
        b, half = divmod(core, 2)
        o = res.results[core]["out"].astype(np.float32)
        for i, g in enumerate(c.CHUNKS[half]):
            out[b, g * P:(g + 1) * P] = o[i * P:(i + 1) * P]
    out += np.asarray(inputs["bh"], np.float32)
    return out
